# revision 23
# baseline (speedup 1.0000x reference)
"""Multi-head causal attention (B=2, S=2048, H=16, D=64) on 8 TRN2 NeuronCores.

Sharding: data-parallel over batch (2) x tensor-parallel over head groups (4).
Core c handles batch b = c // 4, head group g = c % 4 (heads 4g..4g+3).
Each core computes q/k/v projections for its 4 heads, RoPE, causal
flash-style attention (upper-triangular blocks skipped), and a partial
output projection out_partial = attn_out @ Wo[256g:256g+256].  The host
sums the 4 partials per batch and adds the (bias) terms.

Key layout/engine choices:
 - QKV projections run as fp8e4 DoubleRow matmuls (hi+lo split of both x
   and W computed on host; 3 of 4 cross products kept -> ~0.1% error,
   0.75x the PE cycles of fp32r and half the input DMA bytes).
 - q/k are computed TRANSPOSED (d on partitions) with W as the stationary
   operand; Wq/Wk columns are permuted to [all even | all odd] so RoPE
   runs as full-128-partition DVE ops.
 - the eo->head-contiguous regroup is a single SBUF->SBUF DMA per
   128-partition chunk: dst[2p+eo] = src[p, eo] (tandem-linear pairing),
   which interleaves each head's components as [e0 o0 e1 o1 ...] -- a
   permutation applied identically to q and k, so scores are unchanged.
 - scores/PV/out-proj matmuls run in bf16 (1 cyc/row at any N, so causal
   tail blocks are trimmed tightly to the diagonal).
 - causal masking is a gpsimd affine_select zeroing the upper triangle of
   the diagonal 128-col block of exp(scores) (no mask tensor, no DVE add;
   exp of unmasked scores is safe: |scores/8| ~ 5).
 - softmax denominators come free from a ones-column appended to v; the
   reciprocal reads that PSUM row directly and a gpsimd partition
   broadcast fans it out for the (PSUM x SBUF) normalize multiply.
 - output staging is copied PSUM->SBUF on DVE/ACT and stored from the SP
   HWDGE ring (no gpsimd SWDGE engine cost).
"""

import os
import numpy as np
import ml_dtypes
from contextlib import ExitStack

import concourse.bass as bass
import concourse.tile as tile
from concourse import bacc, mybir
from concourse.alu_op_type import AluOpType
from concourse.bass_utils import run_bass_kernel_spmd

F32 = mybir.dt.float32
BF16 = mybir.dt.bfloat16
FP8 = mybir.dt.float8e4
AF = mybir.ActivationFunctionType
DR = mybir.MatmulPerfMode.DoubleRow
E4 = ml_dtypes.float8_e4m3
BF = ml_dtypes.bfloat16

B, S, H, D = 2, 2048, 16, 64
HID = H * D           # 1024
NCORES = 8
G = 4                 # head groups
HPG = H // G          # heads per group = 4
DG = HPG * D          # per-group model dim = 256
KS = HID // 128       # 8 k-subtiles
NQ = 4                # S quarters (chunks of 512)
SB = S // 128         # 16 s-blocks

# fp8 hi-lo pairings: (w hi/lo slot, x hi/lo slot); the lo*lo term is
# dropped (~1e-3 relative contribution)
PAIRS = ((0, 0), (0, 1), (1, 0))

# W entries are ~N(0, 1/HID); scale them up so the hi-lo fp8 residual
# stays above e4m3's smallest subnormal (2^-9).  The 1/WSCALE comes out
# for free: cos/sin are pre-divided (rope multiplies by them) and the v
# copy uses the activation scale.
WSCALE = 64.0

EX_BUFS = 3
STG_BUFS = 3


def build_program():
    nc = bacc.Bacc("TRN2", target_bir_lowering=False, debug=False,
                   num_devices=NCORES)

    xpk = nc.dram_tensor("xpk", [NQ, 128, KS, 2, 512], FP8,
                         kind="ExternalInput").ap()
    wq8 = nc.dram_tensor("wq8", [128, KS, 2, DG], FP8, kind="ExternalInput").ap()
    wk8 = nc.dram_tensor("wk8", [128, KS, 2, DG], FP8, kind="ExternalInput").ap()
    wv8 = nc.dram_tensor("wv8", [128, KS, 2, DG], FP8, kind="ExternalInput").ap()
    wo = nc.dram_tensor("wo", [DG, HID], BF16, kind="ExternalInput").ap()
    bqp = nc.dram_tensor("bqp", [128, 2], F32, kind="ExternalInput").ap()
    bkp = nc.dram_tensor("bkp", [128, 2], F32, kind="ExternalInput").ap()
    cos4 = nc.dram_tensor("cos4", [128, S], F32, kind="ExternalInput").ap()
    sin4 = nc.dram_tensor("sin4", [128, S], F32, kind="ExternalInput").ap()
    out = nc.dram_tensor("out", [SB, 128, HID], F32, kind="ExternalOutput").ap()

    with tile.TileContext(nc) as tc, ExitStack() as ctx:
        const = ctx.enter_context(tc.tile_pool(name="const", bufs=1))
        xp = ctx.enter_context(tc.tile_pool(name="xp", bufs=2))
        tmp = ctx.enter_context(tc.tile_pool(name="tmp", bufs=6))
        ex = ctx.enter_context(tc.tile_pool(name="ex", bufs=EX_BUFS))
        stg = ctx.enter_context(tc.tile_pool(name="stg", bufs=STG_BUFS))
        rcp = ctx.enter_context(tc.tile_pool(name="rcp", bufs=2))
        rbp = ctx.enter_context(tc.tile_pool(name="rbp", bufs=2))
        ps = ctx.enter_context(tc.tile_pool(name="ps", bufs=2, space="PSUM"))
        psc = ctx.enter_context(tc.tile_pool(name="psc", bufs=2, space="PSUM"))
        ppv = ctx.enter_context(tc.tile_pool(name="ppv", bufs=2, space="PSUM"))

        # ---- persistent SBUF tiles ----
        wq_t = const.tile([128, KS, 2, DG], FP8)
        wk_t = const.tile([128, KS, 2, DG], FP8)
        wv_t = const.tile([128, KS, 2, DG], FP8)
        wo_t = const.tile([128, 2, HID], BF16)
        cos_t = const.tile([128, S], F32)
        sin_t = const.tile([128, S], F32)
        bq_t = const.tile([128, 2], F32)
        bk_t = const.tile([128, 2], F32)
        v1_t = const.tile([128, SB, HPG, D + 1], BF16)  # v blocks + ones col
        qr_t = const.tile([128, 2, S], BF16)   # roped q, [evens|odds] chunks
        kr_t = const.tile([128, 2, S], BF16)
        qh_t = const.tile([128, 2, S], BF16)   # head-contiguous roped q
        kh_t = const.tile([128, 2, S], BF16)
        o_t = const.tile([128, 2, S], BF16)    # attn outT (hd on partitions)

        wo_loaded = []
        outr = out  # [SB, 128, HID]

        # early loads, true dependency order (SP HWDGE ring is FIFO);
        # all weights go first so no regroup DMA can park ahead of them.
        # wq/wk come in ktile halves so the first matmuls start sooner.
        nc.sync.dma_start(wq_t[:, 0:4], wq8[:, 0:4])
        nc.sync.dma_start(bq_t[:], bqp)
        nc.sync.dma_start(wk_t[:, 0:4], wk8[:, 0:4])
        nc.sync.dma_start(bk_t[:], bkp)
        nc.sync.dma_start(wq_t[:, 4:8], wq8[:, 4:8])
        nc.sync.dma_start(wk_t[:, 4:8], wk8[:, 4:8])
        nc.sync.dma_start(wv_t[:], wv8)

        def rope_a(pc0, b_t, js):
            """t1=(e+b0)*cos, t3=(e+b0)*sin -- releases the evens psum after
            just two reads so the next projection chunk gets its bank."""
            t1 = tmp.tile([128, 512], F32, name="t1", tag="tt")
            nc.vector.scalar_tensor_tensor(t1[:], pc0[:], b_t[:, 0:1],
                                           cos_t[:, js], AluOpType.add,
                                           AluOpType.mult)
            yield
            t3 = tmp.tile([128, 512], F32, name="t3", tag="tt")
            nc.vector.scalar_tensor_tensor(t3[:], pc0[:], b_t[:, 0:1],
                                           sin_t[:, js], AluOpType.add,
                                           AluOpType.mult)
            yield
            yield (t1, t3)

        def rope_b(pc1, b_t, rr_t, js, t1, t3):
            t2 = tmp.tile([128, 512], F32, name="t2", tag="tt")
            nc.vector.scalar_tensor_tensor(t2[:], pc1[:], b_t[:, 1:2],
                                           sin_t[:, js], AluOpType.add,
                                           AluOpType.mult)
            nc.vector.tensor_sub(rr_t[:, 0, js], t1[:], t2[:])
            yield
            t4 = tmp.tile([128, 512], F32, name="t4", tag="tt")
            nc.vector.scalar_tensor_tensor(t4[:], pc1[:], b_t[:, 1:2],
                                           cos_t[:, js], AluOpType.add,
                                           AluOpType.mult)
            nc.vector.tensor_add(rr_t[:, 1, js], t3[:], t4[:])
            yield

        def fp8_proj(p_out, lhs_w, xq, c, nmax):
            """12 DoubleRow matmuls accumulating (Whi+Wlo)^T (xhi+xlo).
            ktile-half-major so only the first half of x gates the start."""
            nn = 0
            for half in range(2):
                for (wh, xh) in PAIRS:
                    for t in (2 * half, 2 * half + 1):
                        ks = slice(2 * t, 2 * t + 2)
                        nc.tensor.matmul(p_out,
                                         lhs_w[:, ks, wh, bass.ts(c, 128)],
                                         xq[:, ks, xh, :],
                                         start=(nn == 0), stop=(nn == nmax - 1),
                                         perf_mode=DR)
                        nn += 1
                    yield

        def gen_attn(j):
            """Attention for sq-quarter j; yields between pipeline units."""
            js = bass.ts(j, 512)
            if not wo_loaded:
                wo_loaded.append(1)
                nc.gpsimd.dma_start(
                    wo_t[:], wo.rearrange("(o p) n -> p o n", p=128))
            nblk = 4 * j + 4
            for cc in range(2):
                pvs = [ppv.tile([D + 1, 512], F32, name="pv", tag="pv")
                       for _ in range(2)]
                for i in range(nblk):
                    db = 128 * i - 512 * j
                    c0 = max(0, db)      # bf16: no small-N rate penalty
                    n = 512 - c0
                    spb = psc.tile([128, 2, 512], F32, name="sp", tag="sc")
                    for a in range(2):
                        hp = slice(64 * a, 64 * a + 64)
                        nc.tensor.matmul(spb[:, a, :n],
                                         kh_t[hp, cc, bass.ts(i, 128)],
                                         qh_t[hp, cc,
                                              512 * j + c0:512 * (j + 1)],
                                         start=True, stop=True)
                    et = ex.tile([128, 2, 512], BF16, name="et")
                    nc.scalar.activation(et[:, :, :n], spb[:, :, :n],
                                         AF.Exp, scale=0.125)
                    if db >= 0:
                        # zero the upper triangle of the diagonal block
                        nc.gpsimd.affine_select(
                            et[:, :, 0:128], et[:, :, 0:128],
                            pattern=[[0, 2], [1, 128]],
                            compare_op=AluOpType.is_ge, fill=0.0,
                            base=0, channel_multiplier=-1)
                    yield
                    for a in range(2):
                        nc.tensor.matmul(pvs[a][:, c0:512],
                                         v1_t[:, i, 2 * cc + a, :],
                                         et[:, a, :n],
                                         start=(i == 0), stop=(i == nblk - 1))
                    yield
                # softmax denominators live in pv rows D: reciprocal them
                # straight out of PSUM, broadcast across partitions on
                # gpsimd, then normalize pv (PSUM) x recip (SBUF) -> o_t
                rds = []
                for a in range(2):
                    rd = rcp.tile([1, 512], F32, name="rd", tag="rd")
                    nc.vector.reciprocal(rd[:], pvs[a][D:D + 1, :])
                    rds.append(rd)
                yield
                for a in range(2):
                    rb = rbp.tile([64, 512], F32, name="rb", tag="rb")
                    nc.gpsimd.partition_broadcast(rb[:], rds[a][0:1, :],
                                                  channels=64)
                    nc.vector.tensor_mul(o_t[64 * a:64 * a + 64, cc, js],
                                         pvs[a][0:D, :], rb[:])
                    yield

        def gen_outproj(j):
            for sl in range(4):
                sb = 4 * j + sl
                ps0 = ps.tile([128, 512], F32, name="psC0", tag="ps")
                ps1 = ps.tile([128, 512], F32, name="psC1", tag="ps")
                for k in range(2):
                    nc.tensor.matmul(ps0[:], o_t[:, k, bass.ts(sb, 128)],
                                     wo_t[:, k, 0:512],
                                     start=(k == 0), stop=(k == 1))
                for k in range(2):
                    nc.tensor.matmul(ps1[:], o_t[:, k, bass.ts(sb, 128)],
                                     wo_t[:, k, 512:1024],
                                     start=(k == 0), stop=(k == 1))
                st = stg.tile([128, 1024], F32, name="st")
                if j == NQ - 1:   # both ACT and DVE are idle at the tail
                    nc.scalar.activation(st[:, 0:512], ps0[:], AF.Copy)
                else:
                    nc.vector.tensor_copy(st[:, 0:512], ps0[:])
                nc.vector.tensor_copy(st[:, 512:1024], ps1[:])
                yield
                if j == NQ - 1 and sl % 2 == 1:
                    # tail stores alternate rings so they drain in parallel
                    nc.scalar.dma_start(outr[sb], st[:])
                else:
                    nc.sync.dma_start(outr[sb], st[:])
                yield

        def gen_proj(qi):
            """fp8 projections + RoPE + head-regroup for quarter qi."""
            js = bass.ts(qi, 512)
            xq = xp.tile([128, KS, 2, 512], FP8, name="xq")
            eng = nc.scalar if qi == 0 else nc.sync
            nh = 4 if qi == 0 else 2
            for hf in range(nh):
                w = KS // nh
                eng.dma_start(xq[:, w * hf:w * hf + w],
                              xpk[qi][:, w * hf:w * hf + w])
            if qi == 0:
                nc.gpsimd.dma_start(cos_t[:], cos4)
                nc.gpsimd.dma_start(sin_t[:], sin4)
                nc.vector.memset(v1_t[:, :, :, D:D + 1], 1.0)
            # v first: its psum tiles release right after the ACT copy, so
            # the pool rotation warms up without waiting on rope
            for sl in range(4):
                sb = 4 * qi + sl
                p = ps.tile([128, 512], F32, name="psAv", tag="ps")
                nn = 0
                for half in range(2):
                    for (wh, xh) in PAIRS:
                        for t in (2 * half, 2 * half + 1):
                            ks = slice(2 * t, 2 * t + 2)
                            nc.tensor.matmul(p[:, :DG],
                                             xq[:, ks, xh, bass.ts(sl, 128)],
                                             wv_t[:, ks, wh, :],
                                             start=(nn == 0), stop=(nn == 11),
                                             perf_mode=DR)
                            nn += 1
                        yield
                nc.scalar.activation(v1_t[:, sb, :, 0:D],
                                     p[:, :DG].rearrange("p (h d) -> p h d",
                                                         d=D),
                                     AF.Copy, scale=1.0 / WSCALE)
                yield
            # q/k chunks interleaved (q-c0, k-c0, q-c1, k-c1) so each psum
            # buffer's rope reads overlap the next chunk's matmuls and the
            # 2-buf rotation never stalls the PE
            ws = ((wq_t, bq_t, qr_t, qh_t), (wk_t, bk_t, kr_t, kh_t))
            t13 = [None, None]
            for c in range(2):
                for wi, (w_t, b_t, rr_t, hh_t) in enumerate(ws):
                    p = ps.tile([128, 512], F32, name="psA", tag="ps")
                    for _ in fp8_proj(p[:], w_t, xq, c, 12):
                        yield
                    if c == 0:
                        for v in rope_a(p, b_t, js):
                            if v is None:
                                yield
                            else:
                                t13[wi] = v
                    else:
                        for _ in rope_b(p, b_t, rr_t, js, *t13[wi]):
                            yield
                        # tandem-linear regroup DMA: dst[2p+eo] = src[p, eo]
                        for cc in range(2):
                            nc.sync.dma_start(hh_t[:, cc, js],
                                              rr_t[64 * cc:64 * cc + 64, :, js])
                        yield

        def drain(g):
            for _ in g:
                pass

        def weave(primary, *others):
            """Emit `primary` to exhaustion, advancing each (gen, rate) in
            `others` by `rate` units per primary unit so the secondary cover
            spreads across the whole primary window; drain leftovers."""
            others = [[g, r, 0.0] for (g, r) in others if g is not None]
            for _ in primary:
                for o in others:
                    if o[0] is None:
                        continue
                    o[2] += o[1]
                    while o[2] >= 1.0:
                        o[2] -= 1.0
                        try:
                            next(o[0])
                        except StopIteration:
                            o[0] = None
                            break
            for o in others:
                if o[0] is not None:
                    drain(o[0])

        N_PROJ = 38.0   # approx yield counts, for pacing rates
        N_OUTP = 8.0

        drain(gen_proj(0))
        prev_c = None
        for j in range(NQ):
            n_attn = 16.0 * j + 22.0
            weave(gen_attn(j),
                  (gen_proj(j + 1) if j + 1 < NQ else None, N_PROJ / n_attn),
                  (prev_c, N_OUTP / n_attn))
            prev_c = gen_outproj(j)
        drain(prev_c)

    nc.compile()
    return nc


_EO_IDX = None


def _eo_index():
    """Column permutation within one head group: all even components of the
    4 heads first (h-major), then all odd components."""
    global _EO_IDX
    if _EO_IDX is None:
        idx = []
        for eo in (0, 1):
            for h in range(HPG):
                idx.extend(range(64 * h + eo, 64 * h + 64, 2))
        _EO_IDX = np.asarray(idx)
    return _EO_IDX


def _hilo(a):
    hi = a.astype(E4)
    lo = (a - hi.astype(np.float32)).astype(E4)
    return hi, lo


def _pack_w(w):
    """[HID, DG] f32 -> [128, KS, 2, DG] fp8 (hi/lo)."""
    hi, lo = _hilo(w)
    hi = hi.reshape(KS, 128, DG).transpose(1, 0, 2)
    lo = lo.reshape(KS, 128, DG).transpose(1, 0, 2)
    return np.ascontiguousarray(np.stack([hi, lo], axis=2))


def _pack_x(xT):
    """[HID, S] f32 -> [NQ, 128, KS, 2, 512] fp8 (hi/lo)."""
    hi, lo = _hilo(xT)
    hi = hi.reshape(KS, 128, NQ, 512)
    lo = lo.reshape(KS, 128, NQ, 512)
    pk = np.stack([hi, lo], axis=3)            # [KS, 128, NQ, 2, 512]
    return np.ascontiguousarray(pk.transpose(2, 1, 0, 3, 4))


def make_in_maps(x, Wq, bq, Wk, bk, Wv, bv, Wo, bo, mask, freqs_cos, freqs_sin):
    idx = _eo_index()
    f32 = np.float32
    cosT = np.ascontiguousarray(freqs_cos.T, dtype=f32)       # (32, S)
    sinT = np.ascontiguousarray(freqs_sin.T, dtype=f32)
    cos4 = np.tile(cosT, (4, 1)) / f32(WSCALE)                # (128, S)
    sin4 = np.tile(sinT, (4, 1)) / f32(WSCALE)

    Wq = np.asarray(Wq, f32)
    Wk = np.asarray(Wk, f32)
    Wv = np.asarray(Wv, f32)
    Wo = np.asarray(Wo, f32)
    xTs = [np.ascontiguousarray(np.asarray(x[b], f32).T) for b in range(B)]
    xpks = [_pack_x(t) for t in xTs]

    in_maps = []
    for core in range(NCORES):
        b, g = core // G, core % G
        cols = slice(DG * g, DG * (g + 1))
        wq8 = _pack_w(np.ascontiguousarray(Wq[:, cols][:, idx], f32) * WSCALE)
        wk8 = _pack_w(np.ascontiguousarray(Wk[:, cols][:, idx], f32) * WSCALE)
        wv8 = _pack_w(np.ascontiguousarray(Wv[:, cols], f32) * WSCALE)
        wo_g = np.ascontiguousarray(Wo[cols, :]).astype(BF)
        bq_g = np.ascontiguousarray(
            np.asarray(bq, f32)[cols][idx].reshape(2, 128).T) * f32(WSCALE)
        bk_g = np.ascontiguousarray(
            np.asarray(bk, f32)[cols][idx].reshape(2, 128).T) * f32(WSCALE)
        in_maps.append(dict(xpk=xpks[b], wq8=wq8, wk8=wk8, wv8=wv8, wo=wo_g,
                            bqp=bq_g, bkp=bk_g, cos4=cos4, sin4=sin4))
    return in_maps


_NC_CACHE = None
LAST_RESULTS = None


def kernel(**inputs):
    global _NC_CACHE
    if _NC_CACHE is None:
        _NC_CACHE = build_program()
    nc = _NC_CACHE

    inputs = {k: np.asarray(v) for k, v in inputs.items()}
    in_maps = make_in_maps(**inputs)
    kwargs = {}
    if os.environ.get("BASS_TRACE"):
        kwargs = dict(trace=True, trace_cores=list(range(NCORES)),
                      stitch_traces=True)
    res = run_bass_kernel_spmd(nc, in_maps, core_ids=list(range(NCORES)),
                               **kwargs)
    global LAST_RESULTS
    LAST_RESULTS = res

    out = np.zeros((B, S, HID), np.float32)
    for core in range(NCORES):
        out[core // G] += res.results[core]["out"].reshape(S, HID)
    out += inputs["bo"].astype(np.float32)
    out += (inputs["bv"].astype(np.float32) @ inputs["Wo"].astype(np.float32))
    return out


# revision 24
# speedup vs baseline: 1.1050x; 1.1050x over previous
"""Multi-head causal attention (B=2, S=2048, H=16, D=64) on 8 TRN2 NeuronCores.

Sharding: data-parallel over batch (2) x tensor-parallel over head groups (4).
Core c handles batch b = c // 4, head group g = c % 4 (heads 4g..4g+3).
Each core computes q/k/v projections for its 4 heads, RoPE, causal
flash-style attention (upper-triangular blocks skipped), and a partial
output projection out_partial = attn_out @ Wo[256g:256g+256].  The host
sums the 4 partials per batch and adds the (bias) terms.

Key layout/engine choices:
 - QKV projections run as fp8e4 DoubleRow matmuls (hi+lo split of both x
   and W computed on host; 3 of 4 cross products kept -> ~0.1% error,
   0.75x the PE cycles of fp32r and half the input DMA bytes).
 - q/k are computed TRANSPOSED (d on partitions) with W as the stationary
   operand; Wq/Wk columns are permuted to [all even | all odd] so RoPE
   runs as full-128-partition DVE ops.
 - the eo->head-contiguous regroup is a single SBUF->SBUF DMA per
   128-partition chunk: dst[2p+eo] = src[p, eo] (tandem-linear pairing),
   which interleaves each head's components as [e0 o0 e1 o1 ...] -- a
   permutation applied identically to q and k, so scores are unchanged.
 - scores/PV/out-proj matmuls run in bf16 (1 cyc/row at any N, so causal
   tail blocks are trimmed tightly to the diagonal).
 - causal masking is a gpsimd affine_select zeroing the upper triangle of
   the diagonal 128-col block of exp(scores) (no mask tensor, no DVE add;
   exp of unmasked scores is safe: |scores/8| ~ 5).
 - softmax denominators come free from a ones-column appended to v; the
   reciprocal reads that PSUM row directly and a gpsimd partition
   broadcast fans it out for the (PSUM x SBUF) normalize multiply.
 - output staging is copied PSUM->SBUF on DVE/ACT and stored from the SP
   HWDGE ring (no gpsimd SWDGE engine cost).
"""

import os
import numpy as np
import ml_dtypes
from contextlib import ExitStack

import concourse.bass as bass
import concourse.tile as tile
from concourse import bacc, mybir
from concourse.alu_op_type import AluOpType
from concourse.bass_utils import run_bass_kernel_spmd

F32 = mybir.dt.float32
BF16 = mybir.dt.bfloat16
FP8 = mybir.dt.float8e4
AF = mybir.ActivationFunctionType
DR = mybir.MatmulPerfMode.DoubleRow
E4 = ml_dtypes.float8_e4m3
BF = ml_dtypes.bfloat16

B, S, H, D = 2, 2048, 16, 64
HID = H * D           # 1024
NCORES = 8
G = 4                 # head groups
HPG = H // G          # heads per group = 4
DG = HPG * D          # per-group model dim = 256
KS = HID // 128       # 8 k-subtiles
NQ = 4                # S quarters (chunks of 512)
SB = S // 128         # 16 s-blocks

# fp8 hi-lo pairings: (w hi/lo slot, x hi/lo slot); the lo*lo term is
# dropped (~1e-3 relative contribution)
PAIRS = ((0, 0), (0, 1), (1, 0))

# W entries are ~N(0, 1/HID); scale them up so the hi-lo fp8 residual
# stays above e4m3's smallest subnormal (2^-9).  The 1/WSCALE comes out
# for free: cos/sin are pre-divided (rope multiplies by them) and the v
# copy uses the activation scale.
WSCALE = 64.0

EX_BUFS = 3
STG_BUFS = 3


def build_program():
    nc = bacc.Bacc("TRN2", target_bir_lowering=False, debug=False,
                   num_devices=NCORES)

    xpk = nc.dram_tensor("xpk", [NQ, 128, KS, 2, 512], FP8,
                         kind="ExternalInput").ap()
    wq8 = nc.dram_tensor("wq8", [128, KS, 2, DG], FP8, kind="ExternalInput").ap()
    wk8 = nc.dram_tensor("wk8", [128, KS, 2, DG], FP8, kind="ExternalInput").ap()
    wv8 = nc.dram_tensor("wv8", [128, KS, 2, DG], FP8, kind="ExternalInput").ap()
    wo = nc.dram_tensor("wo", [DG, HID], BF16, kind="ExternalInput").ap()
    bqp = nc.dram_tensor("bqp", [128, 2], F32, kind="ExternalInput").ap()
    bkp = nc.dram_tensor("bkp", [128, 2], F32, kind="ExternalInput").ap()
    cos4 = nc.dram_tensor("cos4", [128, S], F32, kind="ExternalInput").ap()
    sin4 = nc.dram_tensor("sin4", [128, S], F32, kind="ExternalInput").ap()
    out = nc.dram_tensor("out", [SB, 128, HID], F32, kind="ExternalOutput").ap()

    with tile.TileContext(nc) as tc, ExitStack() as ctx:
        const = ctx.enter_context(tc.tile_pool(name="const", bufs=1))
        xp = ctx.enter_context(tc.tile_pool(name="xp", bufs=2))
        tmp = ctx.enter_context(tc.tile_pool(name="tmp", bufs=6))
        ex = ctx.enter_context(tc.tile_pool(name="ex", bufs=EX_BUFS))
        stg = ctx.enter_context(tc.tile_pool(name="stg", bufs=STG_BUFS))
        rcp = ctx.enter_context(tc.tile_pool(name="rcp", bufs=2))
        rbp = ctx.enter_context(tc.tile_pool(name="rbp", bufs=2))
        ps = ctx.enter_context(tc.tile_pool(name="ps", bufs=2, space="PSUM"))
        psc = ctx.enter_context(tc.tile_pool(name="psc", bufs=2, space="PSUM"))
        ppv = ctx.enter_context(tc.tile_pool(name="ppv", bufs=2, space="PSUM"))

        # ---- persistent SBUF tiles ----
        wq_t = const.tile([128, KS, 2, DG], FP8)
        wk_t = const.tile([128, KS, 2, DG], FP8)
        wv_t = const.tile([128, KS, 2, DG], FP8)
        wo_t = const.tile([128, 2, HID], BF16)
        cos_t = const.tile([128, S], F32)
        sin_t = const.tile([128, S], F32)
        bq_t = const.tile([128, 2], F32)
        bk_t = const.tile([128, 2], F32)
        v1_t = const.tile([128, SB, HPG, D + 1], BF16)  # v blocks + ones col
        qr_t = const.tile([128, 2, S], BF16)   # roped q, [evens|odds] chunks
        kr_t = const.tile([128, 2, S], BF16)
        qh_t = const.tile([128, 2, S], BF16)   # head-contiguous roped q
        kh_t = const.tile([128, 2, S], BF16)
        o_t = const.tile([128, 2, S], BF16)    # attn outT (hd on partitions)

        wo_loaded = []
        outr = out  # [SB, 128, HID]

        # early loads, true dependency order (SP HWDGE ring is FIFO);
        # all weights go first so no regroup DMA can park ahead of them.
        # wq/wk come in ktile halves so the first matmuls start sooner.
        nc.sync.dma_start(wq_t[:, 0:4], wq8[:, 0:4])
        nc.sync.dma_start(bq_t[:], bqp)
        nc.sync.dma_start(wk_t[:, 0:4], wk8[:, 0:4])
        nc.sync.dma_start(bk_t[:], bkp)
        nc.sync.dma_start(wq_t[:, 4:8], wq8[:, 4:8])
        nc.sync.dma_start(wk_t[:, 4:8], wk8[:, 4:8])
        nc.sync.dma_start(wv_t[:], wv8)

        def rope_a(pc0, b_t, js):
            """t1=(e+b0)*cos, t3=(e+b0)*sin -- releases the evens psum after
            just two reads so the next projection chunk gets its bank."""
            t1 = tmp.tile([128, 512], F32, name="t1", tag="tt")
            nc.vector.scalar_tensor_tensor(t1[:], pc0[:], b_t[:, 0:1],
                                           cos_t[:, js], AluOpType.add,
                                           AluOpType.mult)
            yield
            t3 = tmp.tile([128, 512], F32, name="t3", tag="tt")
            nc.vector.scalar_tensor_tensor(t3[:], pc0[:], b_t[:, 0:1],
                                           sin_t[:, js], AluOpType.add,
                                           AluOpType.mult)
            yield
            yield (t1, t3)

        def rope_b(pc1, b_t, rr_t, js, t1, t3):
            t2 = tmp.tile([128, 512], F32, name="t2", tag="tt")
            nc.vector.scalar_tensor_tensor(t2[:], pc1[:], b_t[:, 1:2],
                                           sin_t[:, js], AluOpType.add,
                                           AluOpType.mult)
            nc.vector.tensor_sub(rr_t[:, 0, js], t1[:], t2[:])
            yield
            t4 = tmp.tile([128, 512], F32, name="t4", tag="tt")
            nc.vector.scalar_tensor_tensor(t4[:], pc1[:], b_t[:, 1:2],
                                           cos_t[:, js], AluOpType.add,
                                           AluOpType.mult)
            nc.vector.tensor_add(rr_t[:, 1, js], t3[:], t4[:])
            yield

        def fp8_proj(p_out, lhs_w, xq, c, nmax):
            """12 DoubleRow matmuls accumulating (Whi+Wlo)^T (xhi+xlo).
            ktile-half-major so only the first half of x gates the start."""
            nn = 0
            for half in range(2):
                for (wh, xh) in PAIRS:
                    for t in (2 * half, 2 * half + 1):
                        ks = slice(2 * t, 2 * t + 2)
                        nc.tensor.matmul(p_out,
                                         lhs_w[:, ks, wh, bass.ts(c, 128)],
                                         xq[:, ks, xh, :],
                                         start=(nn == 0), stop=(nn == nmax - 1),
                                         perf_mode=DR)
                        nn += 1
                    yield

        def gen_attn(j):
            """Attention for sq-quarter j; yields between pipeline units."""
            js = bass.ts(j, 512)
            if not wo_loaded:
                wo_loaded.append(1)
                nc.gpsimd.dma_start(
                    wo_t[:], wo.rearrange("(o p) n -> p o n", p=128))
            nblk = 4 * j + 4
            for cc in range(2):
                pvs = [ppv.tile([D + 1, 512], F32, name="pv", tag="pv")
                       for _ in range(2)]
                for i in range(nblk):
                    db = 128 * i - 512 * j
                    c0 = max(0, db)      # bf16: no small-N rate penalty
                    n = 512 - c0
                    spb = psc.tile([128, 2, 512], F32, name="sp", tag="sc")
                    for a in range(2):
                        hp = slice(64 * a, 64 * a + 64)
                        nc.tensor.matmul(spb[:, a, :n],
                                         kh_t[hp, cc, bass.ts(i, 128)],
                                         qh_t[hp, cc,
                                              512 * j + c0:512 * (j + 1)],
                                         start=True, stop=True)
                    et = ex.tile([128, 2, 512], BF16, name="et")
                    nc.scalar.activation(et[:, :, :n], spb[:, :, :n],
                                         AF.Exp, scale=0.125)
                    if db >= 0:
                        # zero the upper triangle of the diagonal block
                        nc.gpsimd.affine_select(
                            et[:, :, 0:128], et[:, :, 0:128],
                            pattern=[[0, 2], [1, 128]],
                            compare_op=AluOpType.is_ge, fill=0.0,
                            base=0, channel_multiplier=-1)
                    yield
                    for a in range(2):
                        nc.tensor.matmul(pvs[a][:, c0:512],
                                         v1_t[:, i, 2 * cc + a, :],
                                         et[:, a, :n],
                                         start=(i == 0), stop=(i == nblk - 1))
                    yield
                # softmax denominators live in pv rows D: reciprocal them
                # straight out of PSUM, broadcast across partitions on
                # gpsimd, then normalize pv (PSUM) x recip (SBUF) -> o_t
                rds = []
                for a in range(2):
                    rd = rcp.tile([1, 512], F32, name="rd", tag="rd")
                    nc.vector.reciprocal(rd[:], pvs[a][D:D + 1, :])
                    rds.append(rd)
                yield
                for a in range(2):
                    rb = rbp.tile([64, 512], F32, name="rb", tag="rb")
                    nc.gpsimd.partition_broadcast(rb[:], rds[a][0:1, :],
                                                  channels=64)
                    nc.vector.tensor_mul(o_t[64 * a:64 * a + 64, cc, js],
                                         pvs[a][0:D, :], rb[:])
                    yield

        def gen_outproj(j):
            for sl in range(4):
                sb = 4 * j + sl
                ps0 = ps.tile([128, 512], F32, name="psC0", tag="ps")
                ps1 = ps.tile([128, 512], F32, name="psC1", tag="ps")
                for k in range(2):
                    nc.tensor.matmul(ps0[:], o_t[:, k, bass.ts(sb, 128)],
                                     wo_t[:, k, 0:512],
                                     start=(k == 0), stop=(k == 1))
                for k in range(2):
                    nc.tensor.matmul(ps1[:], o_t[:, k, bass.ts(sb, 128)],
                                     wo_t[:, k, 512:1024],
                                     start=(k == 0), stop=(k == 1))
                st = stg.tile([128, 1024], F32, name="st")
                if j == NQ - 1:   # both ACT and DVE are idle at the tail
                    nc.scalar.activation(st[:, 0:512], ps0[:], AF.Copy)
                else:
                    nc.vector.tensor_copy(st[:, 0:512], ps0[:])
                nc.vector.tensor_copy(st[:, 512:1024], ps1[:])
                yield
                if j == NQ - 1 and sl % 2 == 1:
                    # tail stores alternate rings so they drain in parallel
                    nc.scalar.dma_start(outr[sb], st[:])
                else:
                    nc.sync.dma_start(outr[sb], st[:])
                yield

        def gen_proj(qi):
            """fp8 projections + RoPE + head-regroup for quarter qi."""
            js = bass.ts(qi, 512)
            xq = xp.tile([128, KS, 2, 512], FP8, name="xq")
            eng = nc.scalar if qi == 0 else nc.sync
            nh = 4 if qi == 0 else 2
            for hf in range(nh):
                w = KS // nh
                eng.dma_start(xq[:, w * hf:w * hf + w],
                              xpk[qi][:, w * hf:w * hf + w])
            if qi == 0:
                nc.gpsimd.dma_start(cos_t[:], cos4)
                nc.gpsimd.dma_start(sin_t[:], sin4)
                nc.vector.memset(v1_t[:, :, :, D:D + 1], 1.0)
            def gen_v():
                # v psum tiles release right after the ACT copy, so at
                # startup (quarter 0) running v first warms the rotation
                # without waiting on rope; in woven quarters v goes last so
                # its ACT copies don't park ahead of the exp stream
                for sl in range(4):
                    sb = 4 * qi + sl
                    p = ps.tile([128, 512], F32, name="psAv", tag="ps")
                    nn = 0
                    for half in range(2):
                        for (wh, xh) in PAIRS:
                            for t in (2 * half, 2 * half + 1):
                                ks = slice(2 * t, 2 * t + 2)
                                nc.tensor.matmul(p[:, :DG],
                                                 xq[:, ks, xh,
                                                    bass.ts(sl, 128)],
                                                 wv_t[:, ks, wh, :],
                                                 start=(nn == 0),
                                                 stop=(nn == 11),
                                                 perf_mode=DR)
                                nn += 1
                            yield
                    nc.scalar.activation(v1_t[:, sb, :, 0:D],
                                         p[:, :DG].rearrange(
                                             "p (h d) -> p h d", d=D),
                                         AF.Copy, scale=1.0 / WSCALE)
                    yield

            if qi == 0:
                for _ in gen_v():
                    yield
            # q/k chunks interleaved (q-c0, k-c0, q-c1, k-c1) so each psum
            # buffer's rope reads overlap the next chunk's matmuls and the
            # 2-buf rotation never stalls the PE
            ws = ((wq_t, bq_t, qr_t, qh_t), (wk_t, bk_t, kr_t, kh_t))
            t13 = [None, None]
            for c in range(2):
                for wi, (w_t, b_t, rr_t, hh_t) in enumerate(ws):
                    p = ps.tile([128, 512], F32, name="psA", tag="ps")
                    for _ in fp8_proj(p[:], w_t, xq, c, 12):
                        yield
                    if c == 0:
                        for v in rope_a(p, b_t, js):
                            if v is None:
                                yield
                            else:
                                t13[wi] = v
                    else:
                        for _ in rope_b(p, b_t, rr_t, js, *t13[wi]):
                            yield
                        # tandem-linear regroup DMA: dst[2p+eo] = src[p, eo]
                        for cc in range(2):
                            nc.sync.dma_start(hh_t[:, cc, js],
                                              rr_t[64 * cc:64 * cc + 64, :, js])
                        yield
            if qi > 0:
                for _ in gen_v():
                    yield

        def drain(g):
            for _ in g:
                pass

        def weave(primary, *others):
            """Emit `primary` to exhaustion, advancing each (gen, rate) in
            `others` by `rate` units per primary unit so the secondary cover
            spreads across the whole primary window; drain leftovers."""
            others = [[g, r, 0.0] for (g, r) in others if g is not None]
            for _ in primary:
                for o in others:
                    if o[0] is None:
                        continue
                    o[2] += o[1]
                    while o[2] >= 1.0:
                        o[2] -= 1.0
                        try:
                            next(o[0])
                        except StopIteration:
                            o[0] = None
                            break
            for o in others:
                if o[0] is not None:
                    drain(o[0])

        N_PROJ = 38.0   # approx yield counts, for pacing rates
        N_OUTP = 8.0

        drain(gen_proj(0))
        prev_c = None
        for j in range(NQ):
            n_attn = 16.0 * j + 22.0
            weave(gen_attn(j),
                  (gen_proj(j + 1) if j + 1 < NQ else None, N_PROJ / n_attn),
                  (prev_c, N_OUTP / n_attn))
            prev_c = gen_outproj(j)
        drain(prev_c)

    nc.compile()
    return nc


_EO_IDX = None


def _eo_index():
    """Column permutation within one head group: all even components of the
    4 heads first (h-major), then all odd components."""
    global _EO_IDX
    if _EO_IDX is None:
        idx = []
        for eo in (0, 1):
            for h in range(HPG):
                idx.extend(range(64 * h + eo, 64 * h + 64, 2))
        _EO_IDX = np.asarray(idx)
    return _EO_IDX


def _hilo(a):
    hi = a.astype(E4)
    lo = (a - hi.astype(np.float32)).astype(E4)
    return hi, lo


def _pack_w(w):
    """[HID, DG] f32 -> [128, KS, 2, DG] fp8 (hi/lo)."""
    hi, lo = _hilo(w)
    hi = hi.reshape(KS, 128, DG).transpose(1, 0, 2)
    lo = lo.reshape(KS, 128, DG).transpose(1, 0, 2)
    return np.ascontiguousarray(np.stack([hi, lo], axis=2))


def _pack_x(xT):
    """[HID, S] f32 -> [NQ, 128, KS, 2, 512] fp8 (hi/lo)."""
    hi, lo = _hilo(xT)
    hi = hi.reshape(KS, 128, NQ, 512)
    lo = lo.reshape(KS, 128, NQ, 512)
    pk = np.stack([hi, lo], axis=3)            # [KS, 128, NQ, 2, 512]
    return np.ascontiguousarray(pk.transpose(2, 1, 0, 3, 4))


def make_in_maps(x, Wq, bq, Wk, bk, Wv, bv, Wo, bo, mask, freqs_cos, freqs_sin):
    idx = _eo_index()
    f32 = np.float32
    cosT = np.ascontiguousarray(freqs_cos.T, dtype=f32)       # (32, S)
    sinT = np.ascontiguousarray(freqs_sin.T, dtype=f32)
    cos4 = np.tile(cosT, (4, 1)) / f32(WSCALE)                # (128, S)
    sin4 = np.tile(sinT, (4, 1)) / f32(WSCALE)

    Wq = np.asarray(Wq, f32)
    Wk = np.asarray(Wk, f32)
    Wv = np.asarray(Wv, f32)
    Wo = np.asarray(Wo, f32)
    xTs = [np.ascontiguousarray(np.asarray(x[b], f32).T) for b in range(B)]
    xpks = [_pack_x(t) for t in xTs]

    in_maps = []
    for core in range(NCORES):
        b, g = core // G, core % G
        cols = slice(DG * g, DG * (g + 1))
        wq8 = _pack_w(np.ascontiguousarray(Wq[:, cols][:, idx], f32) * WSCALE)
        wk8 = _pack_w(np.ascontiguousarray(Wk[:, cols][:, idx], f32) * WSCALE)
        wv8 = _pack_w(np.ascontiguousarray(Wv[:, cols], f32) * WSCALE)
        wo_g = np.ascontiguousarray(Wo[cols, :]).astype(BF)
        bq_g = np.ascontiguousarray(
            np.asarray(bq, f32)[cols][idx].reshape(2, 128).T) * f32(WSCALE)
        bk_g = np.ascontiguousarray(
            np.asarray(bk, f32)[cols][idx].reshape(2, 128).T) * f32(WSCALE)
        in_maps.append(dict(xpk=xpks[b], wq8=wq8, wk8=wk8, wv8=wv8, wo=wo_g,
                            bqp=bq_g, bkp=bk_g, cos4=cos4, sin4=sin4))
    return in_maps


_NC_CACHE = None
LAST_RESULTS = None


def kernel(**inputs):
    global _NC_CACHE
    if _NC_CACHE is None:
        _NC_CACHE = build_program()
    nc = _NC_CACHE

    inputs = {k: np.asarray(v) for k, v in inputs.items()}
    in_maps = make_in_maps(**inputs)
    kwargs = {}
    if os.environ.get("BASS_TRACE"):
        kwargs = dict(trace=True, trace_cores=list(range(NCORES)),
                      stitch_traces=True)
    res = run_bass_kernel_spmd(nc, in_maps, core_ids=list(range(NCORES)),
                               **kwargs)
    global LAST_RESULTS
    LAST_RESULTS = res

    out = np.zeros((B, S, HID), np.float32)
    for core in range(NCORES):
        out[core // G] += res.results[core]["out"].reshape(S, HID)
    out += inputs["bo"].astype(np.float32)
    out += (inputs["bv"].astype(np.float32) @ inputs["Wo"].astype(np.float32))
    return out


# revision 25
# speedup vs baseline: 1.1230x; 1.0164x over previous
"""Multi-head causal attention (B=2, S=2048, H=16, D=64) on 8 TRN2 NeuronCores.

Sharding: data-parallel over batch (2) x tensor-parallel over head groups (4).
Core c handles batch b = c // 4, head group g = c % 4 (heads 4g..4g+3).
Each core computes q/k/v projections for its 4 heads, RoPE, causal
flash-style attention (upper-triangular blocks skipped), and a partial
output projection out_partial = attn_out @ Wo[256g:256g+256].  The host
sums the 4 partials per batch and adds the (bias) terms.

Key layout/engine choices:
 - QKV projections run as fp8e4 DoubleRow matmuls (hi+lo split of both x
   and W computed on host; 3 of 4 cross products kept -> ~0.1% error,
   0.75x the PE cycles of fp32r and half the input DMA bytes).
 - q/k are computed TRANSPOSED (d on partitions) with W as the stationary
   operand; Wq/Wk columns are permuted to [all even | all odd] so RoPE
   runs as full-128-partition DVE ops.
 - the eo->head-contiguous regroup is a single SBUF->SBUF DMA per
   128-partition chunk: dst[2p+eo] = src[p, eo] (tandem-linear pairing),
   which interleaves each head's components as [e0 o0 e1 o1 ...] -- a
   permutation applied identically to q and k, so scores are unchanged.
 - scores/PV/out-proj matmuls run in bf16 (1 cyc/row at any N, so causal
   tail blocks are trimmed tightly to the diagonal).
 - causal masking is a gpsimd affine_select zeroing the upper triangle of
   the diagonal 128-col block of exp(scores) (no mask tensor, no DVE add;
   exp of unmasked scores is safe: |scores/8| ~ 5).
 - softmax denominators come free from a ones-column appended to v; the
   reciprocal reads that PSUM row directly and a gpsimd partition
   broadcast fans it out for the (PSUM x SBUF) normalize multiply.
 - output staging is copied PSUM->SBUF on DVE/ACT and stored from the SP
   HWDGE ring (no gpsimd SWDGE engine cost).
"""

import os
import numpy as np
import ml_dtypes
from contextlib import ExitStack

import concourse.bass as bass
import concourse.tile as tile
from concourse import bacc, mybir
from concourse.alu_op_type import AluOpType
from concourse.bass_utils import run_bass_kernel_spmd

F32 = mybir.dt.float32
BF16 = mybir.dt.bfloat16
FP8 = mybir.dt.float8e4
AF = mybir.ActivationFunctionType
DR = mybir.MatmulPerfMode.DoubleRow
E4 = ml_dtypes.float8_e4m3
BF = ml_dtypes.bfloat16

B, S, H, D = 2, 2048, 16, 64
HID = H * D           # 1024
NCORES = 8
G = 4                 # head groups
HPG = H // G          # heads per group = 4
DG = HPG * D          # per-group model dim = 256
KS = HID // 128       # 8 k-subtiles
NQ = 4                # S quarters (chunks of 512)
SB = S // 128         # 16 s-blocks

# fp8 hi-lo pairings: (w hi/lo slot, x hi/lo slot); the lo*lo term is
# dropped (~1e-3 relative contribution)
PAIRS = ((0, 0), (0, 1), (1, 0))

# W entries are ~N(0, 1/HID); scale them up so the hi-lo fp8 residual
# stays above e4m3's smallest subnormal (2^-9).  The 1/WSCALE comes out
# for free: cos/sin are pre-divided (rope multiplies by them) and the v
# copy uses the activation scale.
WSCALE = 64.0

EX_BUFS = 3
STG_BUFS = 3


def build_program():
    nc = bacc.Bacc("TRN2", target_bir_lowering=False, debug=False,
                   num_devices=NCORES)

    xpk = nc.dram_tensor("xpk", [NQ, 128, KS, 2, 512], FP8,
                         kind="ExternalInput").ap()
    wq8 = nc.dram_tensor("wq8", [128, KS, 2, DG], FP8, kind="ExternalInput").ap()
    wk8 = nc.dram_tensor("wk8", [128, KS, 2, DG], FP8, kind="ExternalInput").ap()
    wv8 = nc.dram_tensor("wv8", [128, KS, 2, DG], FP8, kind="ExternalInput").ap()
    wo = nc.dram_tensor("wo", [DG, HID], BF16, kind="ExternalInput").ap()
    bqp = nc.dram_tensor("bqp", [128, 2], F32, kind="ExternalInput").ap()
    bkp = nc.dram_tensor("bkp", [128, 2], F32, kind="ExternalInput").ap()
    cos4 = nc.dram_tensor("cos4", [128, S], F32, kind="ExternalInput").ap()
    sin4 = nc.dram_tensor("sin4", [128, S], F32, kind="ExternalInput").ap()
    out = nc.dram_tensor("out", [SB, 128, HID], F32, kind="ExternalOutput").ap()

    with tile.TileContext(nc) as tc, ExitStack() as ctx:
        const = ctx.enter_context(tc.tile_pool(name="const", bufs=1))
        xp = ctx.enter_context(tc.tile_pool(name="xp", bufs=2))
        tmp = ctx.enter_context(tc.tile_pool(name="tmp", bufs=6))
        ex = ctx.enter_context(tc.tile_pool(name="ex", bufs=EX_BUFS))
        stg = ctx.enter_context(tc.tile_pool(name="stg", bufs=STG_BUFS))
        rcp = ctx.enter_context(tc.tile_pool(name="rcp", bufs=2))
        rbp = ctx.enter_context(tc.tile_pool(name="rbp", bufs=2))
        ps = ctx.enter_context(tc.tile_pool(name="ps", bufs=2, space="PSUM"))
        psc = ctx.enter_context(tc.tile_pool(name="psc", bufs=2, space="PSUM"))
        ppv = ctx.enter_context(tc.tile_pool(name="ppv", bufs=2, space="PSUM"))

        # ---- persistent SBUF tiles ----
        wq_t = const.tile([128, KS, 2, DG], FP8)
        wk_t = const.tile([128, KS, 2, DG], FP8)
        wv_t = const.tile([128, KS, 2, DG], FP8)
        wo_t = const.tile([128, 2, HID], BF16)
        cos_t = const.tile([128, S], F32)
        sin_t = const.tile([128, S], F32)
        bq_t = const.tile([128, 2], F32)
        bk_t = const.tile([128, 2], F32)
        v1_t = const.tile([128, SB, HPG, D + 1], BF16)  # v blocks + ones col
        qr_t = const.tile([128, 2, S], BF16)   # roped q, [evens|odds] chunks
        kr_t = const.tile([128, 2, S], BF16)
        qh_t = const.tile([128, 2, S], BF16)   # head-contiguous roped q
        kh_t = const.tile([128, 2, S], BF16)
        o_t = const.tile([128, 2, S], BF16)    # attn outT (hd on partitions)

        wo_loaded = []
        outr = out  # [SB, 128, HID]

        # early loads, true dependency order (SP HWDGE ring is FIFO);
        # all weights go first so no regroup DMA can park ahead of them.
        # wq/wk come in ktile halves so the first matmuls start sooner.
        nc.sync.dma_start(wq_t[:, 0:4], wq8[:, 0:4])
        nc.sync.dma_start(bq_t[:], bqp)
        nc.sync.dma_start(wk_t[:, 0:4], wk8[:, 0:4])
        nc.sync.dma_start(bk_t[:], bkp)
        nc.sync.dma_start(wq_t[:, 4:8], wq8[:, 4:8])
        nc.sync.dma_start(wk_t[:, 4:8], wk8[:, 4:8])
        nc.sync.dma_start(wv_t[:], wv8)

        def rope_a(pc0, b_t, js):
            """t1=(e+b0)*cos, t3=(e+b0)*sin -- releases the evens psum after
            just two reads so the next projection chunk gets its bank."""
            t1 = tmp.tile([128, 512], F32, name="t1", tag="tt")
            nc.vector.scalar_tensor_tensor(t1[:], pc0[:], b_t[:, 0:1],
                                           cos_t[:, js], AluOpType.add,
                                           AluOpType.mult)
            yield
            t3 = tmp.tile([128, 512], F32, name="t3", tag="tt")
            nc.vector.scalar_tensor_tensor(t3[:], pc0[:], b_t[:, 0:1],
                                           sin_t[:, js], AluOpType.add,
                                           AluOpType.mult)
            yield
            yield (t1, t3)

        def rope_b(pc1, b_t, rr_t, js, t1, t3):
            t2 = tmp.tile([128, 512], F32, name="t2", tag="tt")
            nc.vector.scalar_tensor_tensor(t2[:], pc1[:], b_t[:, 1:2],
                                           sin_t[:, js], AluOpType.add,
                                           AluOpType.mult)
            nc.vector.tensor_sub(rr_t[:, 0, js], t1[:], t2[:])
            yield
            t4 = tmp.tile([128, 512], F32, name="t4", tag="tt")
            nc.vector.scalar_tensor_tensor(t4[:], pc1[:], b_t[:, 1:2],
                                           cos_t[:, js], AluOpType.add,
                                           AluOpType.mult)
            nc.vector.tensor_add(rr_t[:, 1, js], t3[:], t4[:])
            yield

        def fp8_proj(p_out, lhs_w, xq, c, nmax):
            """12 DoubleRow matmuls accumulating (Whi+Wlo)^T (xhi+xlo).
            ktile-half-major so only the first half of x gates the start."""
            nn = 0
            for half in range(2):
                for (wh, xh) in PAIRS:
                    for t in (2 * half, 2 * half + 1):
                        ks = slice(2 * t, 2 * t + 2)
                        nc.tensor.matmul(p_out,
                                         lhs_w[:, ks, wh, bass.ts(c, 128)],
                                         xq[:, ks, xh, :],
                                         start=(nn == 0), stop=(nn == nmax - 1),
                                         perf_mode=DR)
                        nn += 1
                    yield

        def gen_attn(j):
            """Attention for sq-quarter j; yields between pipeline units."""
            js = bass.ts(j, 512)
            if not wo_loaded:
                wo_loaded.append(1)
                nc.gpsimd.dma_start(
                    wo_t[:], wo.rearrange("(o p) n -> p o n", p=128))
            nblk = 4 * j + 4
            for cc in range(2):
                pvs = [ppv.tile([D + 1, 512], F32, name="pv", tag="pv")
                       for _ in range(2)]
                for i in range(nblk):
                    db = 128 * i - 512 * j
                    c0 = max(0, db)      # bf16: no small-N rate penalty
                    n = 512 - c0
                    spb = psc.tile([128, 2, 512], F32, name="sp", tag="sc")
                    for a in range(2):
                        hp = slice(64 * a, 64 * a + 64)
                        nc.tensor.matmul(spb[:, a, :n],
                                         kh_t[hp, cc, bass.ts(i, 128)],
                                         qh_t[hp, cc,
                                              512 * j + c0:512 * (j + 1)],
                                         start=True, stop=True)
                    et = ex.tile([128, 2, 512], BF16, name="et")
                    nc.scalar.activation(et[:, :, :n], spb[:, :, :n],
                                         AF.Exp, scale=0.125)
                    if db >= 0:
                        # zero the upper triangle of the diagonal block
                        nc.gpsimd.affine_select(
                            et[:, :, 0:128], et[:, :, 0:128],
                            pattern=[[0, 2], [1, 128]],
                            compare_op=AluOpType.is_ge, fill=0.0,
                            base=0, channel_multiplier=-1)
                    yield
                    for a in range(2):
                        nc.tensor.matmul(pvs[a][:, c0:512],
                                         v1_t[:, i, 2 * cc + a, :],
                                         et[:, a, :n],
                                         start=(i == 0), stop=(i == nblk - 1))
                    yield
                # softmax denominators live in pv rows D: reciprocal them
                # straight out of PSUM, broadcast across partitions on
                # gpsimd, then normalize pv (PSUM) x recip (SBUF) -> o_t
                rds = []
                for a in range(2):
                    rd = rcp.tile([1, 512], F32, name="rd", tag="rd")
                    nc.vector.reciprocal(rd[:], pvs[a][D:D + 1, :])
                    rds.append(rd)
                yield
                for a in range(2):
                    rb = rbp.tile([64, 512], F32, name="rb", tag="rb")
                    nc.gpsimd.partition_broadcast(rb[:], rds[a][0:1, :],
                                                  channels=64)
                    nc.vector.tensor_mul(o_t[64 * a:64 * a + 64, cc, js],
                                         pvs[a][0:D, :], rb[:])
                    yield

        def gen_outproj(j):
            for sl in range(4):
                sb = 4 * j + sl
                ps0 = ps.tile([128, 512], F32, name="psC0", tag="ps")
                ps1 = ps.tile([128, 512], F32, name="psC1", tag="ps")
                for k in range(2):
                    nc.tensor.matmul(ps0[:], o_t[:, k, bass.ts(sb, 128)],
                                     wo_t[:, k, 0:512],
                                     start=(k == 0), stop=(k == 1))
                for k in range(2):
                    nc.tensor.matmul(ps1[:], o_t[:, k, bass.ts(sb, 128)],
                                     wo_t[:, k, 512:1024],
                                     start=(k == 0), stop=(k == 1))
                st = stg.tile([128, 1024], F32, name="st")
                if j == NQ - 1:   # both ACT and DVE are idle at the tail
                    nc.scalar.activation(st[:, 0:512], ps0[:], AF.Copy)
                else:
                    nc.vector.tensor_copy(st[:, 0:512], ps0[:])
                nc.vector.tensor_copy(st[:, 512:1024], ps1[:])
                yield
                if j == NQ - 1 and sl % 2 == 1:
                    # tail stores alternate rings so they drain in parallel
                    nc.scalar.dma_start(outr[sb], st[:])
                else:
                    nc.sync.dma_start(outr[sb], st[:])
                yield

        def gen_proj(qi):
            """fp8 projections + RoPE + head-regroup for quarter qi."""
            js = bass.ts(qi, 512)
            xq = xp.tile([128, KS, 2, 512], FP8, name="xq")
            eng = nc.scalar if qi == 0 else nc.sync
            nh = 4 if qi == 0 else 2
            for hf in range(nh):
                w = KS // nh
                eng.dma_start(xq[:, w * hf:w * hf + w],
                              xpk[qi][:, w * hf:w * hf + w])
            if qi == 0:
                nc.gpsimd.dma_start(cos_t[:], cos4)
                nc.gpsimd.dma_start(sin_t[:], sin4)
                nc.vector.memset(v1_t[:, :, :, D:D + 1], 1.0)
            def gen_v():
                # v psum tiles release right after the ACT copy, so at
                # startup (quarter 0) running v first warms the rotation
                # without waiting on rope; in woven quarters v goes last so
                # its ACT copies don't park ahead of the exp stream
                for sl in range(4):
                    sb = 4 * qi + sl
                    p = ps.tile([128, 512], F32, name="psAv", tag="ps")
                    nn = 0
                    for half in range(2):
                        for (wh, xh) in PAIRS:
                            for t in (2 * half, 2 * half + 1):
                                ks = slice(2 * t, 2 * t + 2)
                                nc.tensor.matmul(p[:, :DG],
                                                 xq[:, ks, xh,
                                                    bass.ts(sl, 128)],
                                                 wv_t[:, ks, wh, :],
                                                 start=(nn == 0),
                                                 stop=(nn == 11),
                                                 perf_mode=DR)
                                nn += 1
                            yield
                    nc.scalar.activation(v1_t[:, sb, :, 0:D],
                                         p[:, :DG].rearrange(
                                             "p (h d) -> p h d", d=D),
                                         AF.Copy, scale=1.0 / WSCALE)
                    yield

            # q/k chunks interleaved (q-c0, k-c0, q-c1, k-c1) so each psum
            # buffer's rope reads overlap the next chunk's matmuls and the
            # 2-buf rotation never stalls the PE
            ws = ((wq_t, bq_t, qr_t, qh_t), (wk_t, bk_t, kr_t, kh_t))
            t13 = [None, None]
            for c in range(2):
                for wi, (w_t, b_t, rr_t, hh_t) in enumerate(ws):
                    p = ps.tile([128, 512], F32, name="psA", tag="ps")
                    for _ in fp8_proj(p[:], w_t, xq, c, 12):
                        yield
                    if c == 0:
                        for v in rope_a(p, b_t, js):
                            if v is None:
                                yield
                            else:
                                t13[wi] = v
                    else:
                        for _ in rope_b(p, b_t, rr_t, js, *t13[wi]):
                            yield
                        # tandem-linear regroup DMA: dst[2p+eo] = src[p, eo]
                        for cc in range(2):
                            nc.sync.dma_start(hh_t[:, cc, js],
                                              rr_t[64 * cc:64 * cc + 64, :, js])
                        yield
            for _ in gen_v():
                yield

        def drain(g):
            for _ in g:
                pass

        def weave(primary, *others):
            """Emit `primary` to exhaustion, advancing each (gen, rate) in
            `others` by `rate` units per primary unit so the secondary cover
            spreads across the whole primary window; drain leftovers."""
            others = [[g, r, 0.0] for (g, r) in others if g is not None]
            for _ in primary:
                for o in others:
                    if o[0] is None:
                        continue
                    o[2] += o[1]
                    while o[2] >= 1.0:
                        o[2] -= 1.0
                        try:
                            next(o[0])
                        except StopIteration:
                            o[0] = None
                            break
            for o in others:
                if o[0] is not None:
                    drain(o[0])

        N_PROJ = 38.0   # approx yield counts, for pacing rates
        N_OUTP = 8.0

        drain(gen_proj(0))
        prev_c = None
        for j in range(NQ):
            n_attn = 16.0 * j + 22.0
            weave(gen_attn(j),
                  (gen_proj(j + 1) if j + 1 < NQ else None, N_PROJ / n_attn),
                  (prev_c, N_OUTP / n_attn))
            prev_c = gen_outproj(j)
        drain(prev_c)

    nc.compile()
    return nc


_EO_IDX = None


def _eo_index():
    """Column permutation within one head group: all even components of the
    4 heads first (h-major), then all odd components."""
    global _EO_IDX
    if _EO_IDX is None:
        idx = []
        for eo in (0, 1):
            for h in range(HPG):
                idx.extend(range(64 * h + eo, 64 * h + 64, 2))
        _EO_IDX = np.asarray(idx)
    return _EO_IDX


def _hilo(a):
    hi = a.astype(E4)
    lo = (a - hi.astype(np.float32)).astype(E4)
    return hi, lo


def _pack_w(w):
    """[HID, DG] f32 -> [128, KS, 2, DG] fp8 (hi/lo)."""
    hi, lo = _hilo(w)
    hi = hi.reshape(KS, 128, DG).transpose(1, 0, 2)
    lo = lo.reshape(KS, 128, DG).transpose(1, 0, 2)
    return np.ascontiguousarray(np.stack([hi, lo], axis=2))


def _pack_x(xT):
    """[HID, S] f32 -> [NQ, 128, KS, 2, 512] fp8 (hi/lo)."""
    hi, lo = _hilo(xT)
    hi = hi.reshape(KS, 128, NQ, 512)
    lo = lo.reshape(KS, 128, NQ, 512)
    pk = np.stack([hi, lo], axis=3)            # [KS, 128, NQ, 2, 512]
    return np.ascontiguousarray(pk.transpose(2, 1, 0, 3, 4))


def make_in_maps(x, Wq, bq, Wk, bk, Wv, bv, Wo, bo, mask, freqs_cos, freqs_sin):
    idx = _eo_index()
    f32 = np.float32
    cosT = np.ascontiguousarray(freqs_cos.T, dtype=f32)       # (32, S)
    sinT = np.ascontiguousarray(freqs_sin.T, dtype=f32)
    cos4 = np.tile(cosT, (4, 1)) / f32(WSCALE)                # (128, S)
    sin4 = np.tile(sinT, (4, 1)) / f32(WSCALE)

    Wq = np.asarray(Wq, f32)
    Wk = np.asarray(Wk, f32)
    Wv = np.asarray(Wv, f32)
    Wo = np.asarray(Wo, f32)
    xTs = [np.ascontiguousarray(np.asarray(x[b], f32).T) for b in range(B)]
    xpks = [_pack_x(t) for t in xTs]

    in_maps = []
    for core in range(NCORES):
        b, g = core // G, core % G
        cols = slice(DG * g, DG * (g + 1))
        wq8 = _pack_w(np.ascontiguousarray(Wq[:, cols][:, idx], f32) * WSCALE)
        wk8 = _pack_w(np.ascontiguousarray(Wk[:, cols][:, idx], f32) * WSCALE)
        wv8 = _pack_w(np.ascontiguousarray(Wv[:, cols], f32) * WSCALE)
        wo_g = np.ascontiguousarray(Wo[cols, :]).astype(BF)
        bq_g = np.ascontiguousarray(
            np.asarray(bq, f32)[cols][idx].reshape(2, 128).T) * f32(WSCALE)
        bk_g = np.ascontiguousarray(
            np.asarray(bk, f32)[cols][idx].reshape(2, 128).T) * f32(WSCALE)
        in_maps.append(dict(xpk=xpks[b], wq8=wq8, wk8=wk8, wv8=wv8, wo=wo_g,
                            bqp=bq_g, bkp=bk_g, cos4=cos4, sin4=sin4))
    return in_maps


_NC_CACHE = None
LAST_RESULTS = None


def kernel(**inputs):
    global _NC_CACHE
    if _NC_CACHE is None:
        _NC_CACHE = build_program()
    nc = _NC_CACHE

    inputs = {k: np.asarray(v) for k, v in inputs.items()}
    in_maps = make_in_maps(**inputs)
    kwargs = {}
    if os.environ.get("BASS_TRACE"):
        kwargs = dict(trace=True, trace_cores=list(range(NCORES)),
                      stitch_traces=True)
    res = run_bass_kernel_spmd(nc, in_maps, core_ids=list(range(NCORES)),
                               **kwargs)
    global LAST_RESULTS
    LAST_RESULTS = res

    out = np.zeros((B, S, HID), np.float32)
    for core in range(NCORES):
        out[core // G] += res.results[core]["out"].reshape(S, HID)
    out += inputs["bo"].astype(np.float32)
    out += (inputs["bv"].astype(np.float32) @ inputs["Wo"].astype(np.float32))
    return out


# revision 26
# speedup vs baseline: 1.1334x; 1.0092x over previous
"""Multi-head causal attention (B=2, S=2048, H=16, D=64) on 8 TRN2 NeuronCores.

Sharding: data-parallel over batch (2) x tensor-parallel over head groups (4).
Core c handles batch b = c // 4, head group g = c % 4 (heads 4g..4g+3).
Each core computes q/k/v projections for its 4 heads, RoPE, causal
flash-style attention (upper-triangular blocks skipped), and a partial
output projection out_partial = attn_out @ Wo[256g:256g+256].  The host
sums the 4 partials per batch and adds the (bias) terms.

Key layout/engine choices:
 - QKV projections run as fp8e4 DoubleRow matmuls (hi+lo split of both x
   and W computed on host; 3 of 4 cross products kept -> ~0.1% error,
   0.75x the PE cycles of fp32r and half the input DMA bytes).
 - q/k are computed TRANSPOSED (d on partitions) with W as the stationary
   operand; Wq/Wk columns are permuted to [all even | all odd] so RoPE
   runs as full-128-partition DVE ops.
 - the eo->head-contiguous regroup is a single SBUF->SBUF DMA per
   128-partition chunk: dst[2p+eo] = src[p, eo] (tandem-linear pairing),
   which interleaves each head's components as [e0 o0 e1 o1 ...] -- a
   permutation applied identically to q and k, so scores are unchanged.
 - scores/PV/out-proj matmuls run in bf16 (1 cyc/row at any N, so causal
   tail blocks are trimmed tightly to the diagonal).
 - causal masking is a gpsimd affine_select zeroing the upper triangle of
   the diagonal 128-col block of exp(scores) (no mask tensor, no DVE add;
   exp of unmasked scores is safe: |scores/8| ~ 5).
 - softmax denominators come free from a ones-column appended to v; the
   reciprocal reads that PSUM row directly and a gpsimd partition
   broadcast fans it out for the (PSUM x SBUF) normalize multiply.
 - output staging is copied PSUM->SBUF on DVE/ACT and stored from the SP
   HWDGE ring (no gpsimd SWDGE engine cost).
"""

import os
import numpy as np
import ml_dtypes
from contextlib import ExitStack

import concourse.bass as bass
import concourse.tile as tile
from concourse import bacc, mybir
from concourse.alu_op_type import AluOpType
from concourse.bass_utils import run_bass_kernel_spmd

F32 = mybir.dt.float32
BF16 = mybir.dt.bfloat16
FP8 = mybir.dt.float8e4
AF = mybir.ActivationFunctionType
DR = mybir.MatmulPerfMode.DoubleRow
E4 = ml_dtypes.float8_e4m3
BF = ml_dtypes.bfloat16

B, S, H, D = 2, 2048, 16, 64
HID = H * D           # 1024
NCORES = 8
G = 4                 # head groups
HPG = H // G          # heads per group = 4
DG = HPG * D          # per-group model dim = 256
KS = HID // 128       # 8 k-subtiles
NQ = 4                # S quarters (chunks of 512)
SB = S // 128         # 16 s-blocks

# fp8 hi-lo pairings: (w hi/lo slot, x hi/lo slot); the lo*lo term is
# dropped (~1e-3 relative contribution)
PAIRS = ((0, 0), (0, 1), (1, 0))

# W entries are ~N(0, 1/HID); scale them up so the hi-lo fp8 residual
# stays above e4m3's smallest subnormal (2^-9).  The 1/WSCALE comes out
# for free: cos/sin are pre-divided (rope multiplies by them) and the v
# copy uses the activation scale.
WSCALE = 64.0

EX_BUFS = 3
STG_BUFS = 3


def build_program():
    nc = bacc.Bacc("TRN2", target_bir_lowering=False, debug=False,
                   num_devices=NCORES)

    xpk = nc.dram_tensor("xpk", [NQ, 128, KS, 2, 512], FP8,
                         kind="ExternalInput").ap()
    wq8 = nc.dram_tensor("wq8", [128, KS, 2, DG], FP8, kind="ExternalInput").ap()
    wk8 = nc.dram_tensor("wk8", [128, KS, 2, DG], FP8, kind="ExternalInput").ap()
    wv8 = nc.dram_tensor("wv8", [128, KS, 2, DG], FP8, kind="ExternalInput").ap()
    wo = nc.dram_tensor("wo", [DG, HID], BF16, kind="ExternalInput").ap()
    bqp = nc.dram_tensor("bqp", [128, 2], F32, kind="ExternalInput").ap()
    bkp = nc.dram_tensor("bkp", [128, 2], F32, kind="ExternalInput").ap()
    cos4 = nc.dram_tensor("cos4", [128, S], F32, kind="ExternalInput").ap()
    sin4 = nc.dram_tensor("sin4", [128, S], F32, kind="ExternalInput").ap()
    out = nc.dram_tensor("out", [SB, 128, HID], F32, kind="ExternalOutput").ap()

    with tile.TileContext(nc) as tc, ExitStack() as ctx:
        const = ctx.enter_context(tc.tile_pool(name="const", bufs=1))
        xp = ctx.enter_context(tc.tile_pool(name="xp", bufs=2))
        tmp = ctx.enter_context(tc.tile_pool(name="tmp", bufs=6))
        ex = ctx.enter_context(tc.tile_pool(name="ex", bufs=EX_BUFS))
        stg = ctx.enter_context(tc.tile_pool(name="stg", bufs=STG_BUFS))
        rcp = ctx.enter_context(tc.tile_pool(name="rcp", bufs=2))
        rbp = ctx.enter_context(tc.tile_pool(name="rbp", bufs=2))
        ps = ctx.enter_context(tc.tile_pool(name="ps", bufs=2, space="PSUM"))
        psc = ctx.enter_context(tc.tile_pool(name="psc", bufs=2, space="PSUM"))
        ppv = ctx.enter_context(tc.tile_pool(name="ppv", bufs=2, space="PSUM"))

        # ---- persistent SBUF tiles ----
        wq_t = const.tile([128, KS, 2, DG], FP8)
        wk_t = const.tile([128, KS, 2, DG], FP8)
        wv_t = const.tile([128, KS, 2, DG], FP8)
        wo_t = const.tile([128, 2, HID], BF16)
        cos_t = const.tile([128, S], F32)
        sin_t = const.tile([128, S], F32)
        bq_t = const.tile([128, 2], F32)
        bk_t = const.tile([128, 2], F32)
        v1_t = const.tile([128, SB, HPG, D + 1], BF16)  # v blocks + ones col
        qr_t = const.tile([128, 2, S], BF16)   # roped q, [evens|odds] chunks
        kr_t = const.tile([128, 2, S], BF16)
        qh_t = const.tile([128, 2, S], BF16)   # head-contiguous roped q
        kh_t = const.tile([128, 2, S], BF16)
        o_t = const.tile([128, 2, S], BF16)    # attn outT (hd on partitions)

        wo_loaded = []
        outr = out  # [SB, 128, HID]

        # early loads, true dependency order (SP HWDGE ring is FIFO);
        # all weights go first so no regroup DMA can park ahead of them.
        # wq/wk come in ktile halves so the first matmuls start sooner.
        nc.sync.dma_start(wq_t[:, 0:4], wq8[:, 0:4])
        nc.sync.dma_start(bq_t[:], bqp)
        nc.sync.dma_start(wk_t[:, 0:4], wk8[:, 0:4])
        nc.sync.dma_start(bk_t[:], bkp)
        nc.sync.dma_start(wq_t[:, 4:8], wq8[:, 4:8])
        nc.sync.dma_start(wk_t[:, 4:8], wk8[:, 4:8])
        nc.sync.dma_start(wv_t[:], wv8)

        def rope_a(pc0, b_t, js):
            """t1=(e+b0)*cos, t3=(e+b0)*sin -- releases the evens psum after
            just two reads so the next projection chunk gets its bank."""
            t1 = tmp.tile([128, 512], F32, name="t1", tag="tt")
            nc.vector.scalar_tensor_tensor(t1[:], pc0[:], b_t[:, 0:1],
                                           cos_t[:, js], AluOpType.add,
                                           AluOpType.mult)
            yield
            t3 = tmp.tile([128, 512], F32, name="t3", tag="tt")
            nc.vector.scalar_tensor_tensor(t3[:], pc0[:], b_t[:, 0:1],
                                           sin_t[:, js], AluOpType.add,
                                           AluOpType.mult)
            yield
            yield (t1, t3)

        def rope_b(pc1, b_t, rr_t, js, t1, t3):
            t2 = tmp.tile([128, 512], F32, name="t2", tag="tt")
            nc.vector.scalar_tensor_tensor(t2[:], pc1[:], b_t[:, 1:2],
                                           sin_t[:, js], AluOpType.add,
                                           AluOpType.mult)
            nc.vector.tensor_sub(rr_t[:, 0, js], t1[:], t2[:])
            yield
            t4 = tmp.tile([128, 512], F32, name="t4", tag="tt")
            nc.vector.scalar_tensor_tensor(t4[:], pc1[:], b_t[:, 1:2],
                                           cos_t[:, js], AluOpType.add,
                                           AluOpType.mult)
            nc.vector.tensor_add(rr_t[:, 1, js], t3[:], t4[:])
            yield

        def fp8_proj(p_out, lhs_w, xq, c, nmax):
            """12 DoubleRow matmuls accumulating (Whi+Wlo)^T (xhi+xlo).
            ktile-half-major so only the first half of x gates the start."""
            nn = 0
            for half in range(2):
                for (wh, xh) in PAIRS:
                    for t in (2 * half, 2 * half + 1):
                        ks = slice(2 * t, 2 * t + 2)
                        nc.tensor.matmul(p_out,
                                         lhs_w[:, ks, wh, bass.ts(c, 128)],
                                         xq[:, ks, xh, :],
                                         start=(nn == 0), stop=(nn == nmax - 1),
                                         perf_mode=DR)
                        nn += 1
                    yield

        def gen_attn(j):
            """Attention for sq-quarter j; yields between pipeline units."""
            js = bass.ts(j, 512)
            if not wo_loaded:
                wo_loaded.append(1)
                nc.gpsimd.dma_start(
                    wo_t[:], wo.rearrange("(o p) n -> p o n", p=128))
            nblk = 4 * j + 4
            for cc in range(2):
                pvs = [ppv.tile([D + 1, 512], F32, name="pv", tag="pv")
                       for _ in range(2)]
                for i in range(nblk):
                    db = 128 * i - 512 * j
                    c0 = max(0, db)      # bf16: no small-N rate penalty
                    n = 512 - c0
                    spb = psc.tile([128, 2, 512], F32, name="sp", tag="sc")
                    for a in range(2):
                        hp = slice(64 * a, 64 * a + 64)
                        nc.tensor.matmul(spb[:, a, :n],
                                         kh_t[hp, cc, bass.ts(i, 128)],
                                         qh_t[hp, cc,
                                              512 * j + c0:512 * (j + 1)],
                                         start=True, stop=True)
                    et = ex.tile([128, 2, 512], BF16, name="et")
                    nc.scalar.activation(et[:, :, :n], spb[:, :, :n],
                                         AF.Exp, scale=0.125)
                    if db >= 0:
                        # zero the upper triangle of the diagonal block
                        nc.gpsimd.affine_select(
                            et[:, :, 0:128], et[:, :, 0:128],
                            pattern=[[0, 2], [1, 128]],
                            compare_op=AluOpType.is_ge, fill=0.0,
                            base=0, channel_multiplier=-1)
                    yield
                    for a in range(2):
                        nc.tensor.matmul(pvs[a][:, c0:512],
                                         v1_t[:, i, 2 * cc + a, :],
                                         et[:, a, :n],
                                         start=(i == 0), stop=(i == nblk - 1))
                    yield
                # softmax denominators live in pv rows D: reciprocal them
                # straight out of PSUM, broadcast across partitions on
                # gpsimd, then normalize pv (PSUM) x recip (SBUF) -> o_t
                rds = []
                for a in range(2):
                    rd = rcp.tile([1, 512], F32, name="rd", tag="rd")
                    nc.vector.reciprocal(rd[:], pvs[a][D:D + 1, :])
                    rds.append(rd)
                yield
                for a in range(2):
                    rb = rbp.tile([64, 512], F32, name="rb", tag="rb")
                    nc.gpsimd.partition_broadcast(rb[:], rds[a][0:1, :],
                                                  channels=64)
                    nc.vector.tensor_mul(o_t[64 * a:64 * a + 64, cc, js],
                                         pvs[a][0:D, :], rb[:])
                    yield

        def gen_outproj(j):
            for sl in range(4):
                sb = 4 * j + sl
                ps0 = ps.tile([128, 512], F32, name="psC0", tag="ps")
                ps1 = ps.tile([128, 512], F32, name="psC1", tag="ps")
                for k in range(2):
                    nc.tensor.matmul(ps0[:], o_t[:, k, bass.ts(sb, 128)],
                                     wo_t[:, k, 0:512],
                                     start=(k == 0), stop=(k == 1))
                for k in range(2):
                    nc.tensor.matmul(ps1[:], o_t[:, k, bass.ts(sb, 128)],
                                     wo_t[:, k, 512:1024],
                                     start=(k == 0), stop=(k == 1))
                st = stg.tile([128, 1024], F32, name="st")
                if j == NQ - 1:   # both ACT and DVE are idle at the tail
                    nc.scalar.activation(st[:, 0:512], ps0[:], AF.Copy)
                else:
                    nc.vector.tensor_copy(st[:, 0:512], ps0[:])
                nc.vector.tensor_copy(st[:, 512:1024], ps1[:])
                yield
                if j == NQ - 1 and sl == 3:
                    # final store: halves on two rings to shorten the tail
                    nc.scalar.dma_start(outr[sb][:, 0:512], st[:, 0:512])
                    nc.sync.dma_start(outr[sb][:, 512:1024], st[:, 512:1024])
                else:
                    nc.sync.dma_start(outr[sb], st[:])
                yield

        def gen_proj(qi):
            """fp8 projections + RoPE + head-regroup for quarter qi."""
            js = bass.ts(qi, 512)
            xq = xp.tile([128, KS, 2, 512], FP8, name="xq")
            eng = nc.scalar if qi == 0 else nc.sync
            nh = 4 if qi == 0 else 2
            for hf in range(nh):
                w = KS // nh
                eng.dma_start(xq[:, w * hf:w * hf + w],
                              xpk[qi][:, w * hf:w * hf + w])
            if qi == 0:
                nc.gpsimd.dma_start(cos_t[:], cos4)
                nc.gpsimd.dma_start(sin_t[:], sin4)
                nc.vector.memset(v1_t[:, :, :, D:D + 1], 1.0)
            def gen_v():
                # v psum tiles release right after the ACT copy, so at
                # startup (quarter 0) running v first warms the rotation
                # without waiting on rope; in woven quarters v goes last so
                # its ACT copies don't park ahead of the exp stream
                for sl in range(4):
                    sb = 4 * qi + sl
                    p = ps.tile([128, 512], F32, name="psAv", tag="ps")
                    nn = 0
                    for half in range(2):
                        for (wh, xh) in PAIRS:
                            for t in (2 * half, 2 * half + 1):
                                ks = slice(2 * t, 2 * t + 2)
                                nc.tensor.matmul(p[:, :DG],
                                                 xq[:, ks, xh,
                                                    bass.ts(sl, 128)],
                                                 wv_t[:, ks, wh, :],
                                                 start=(nn == 0),
                                                 stop=(nn == 11),
                                                 perf_mode=DR)
                                nn += 1
                            yield
                    nc.scalar.activation(v1_t[:, sb, :, 0:D],
                                         p[:, :DG].rearrange(
                                             "p (h d) -> p h d", d=D),
                                         AF.Copy, scale=1.0 / WSCALE)
                    yield

            # q/k chunks interleaved (q-c0, k-c0, q-c1, k-c1) so each psum
            # buffer's rope reads overlap the next chunk's matmuls and the
            # 2-buf rotation never stalls the PE
            ws = ((wq_t, bq_t, qr_t, qh_t), (wk_t, bk_t, kr_t, kh_t))
            t13 = [None, None]
            for c in range(2):
                for wi, (w_t, b_t, rr_t, hh_t) in enumerate(ws):
                    p = ps.tile([128, 512], F32, name="psA", tag="ps")
                    for _ in fp8_proj(p[:], w_t, xq, c, 12):
                        yield
                    if c == 0:
                        for v in rope_a(p, b_t, js):
                            if v is None:
                                yield
                            else:
                                t13[wi] = v
                    else:
                        for _ in rope_b(p, b_t, rr_t, js, *t13[wi]):
                            yield
                        # tandem-linear regroup DMA: dst[2p+eo] = src[p, eo]
                        for cc in range(2):
                            nc.sync.dma_start(hh_t[:, cc, js],
                                              rr_t[64 * cc:64 * cc + 64, :, js])
                        yield
            for _ in gen_v():
                yield

        def drain(g):
            for _ in g:
                pass

        def weave(primary, *others):
            """Emit `primary` to exhaustion, advancing each (gen, rate) in
            `others` by `rate` units per primary unit so the secondary cover
            spreads across the whole primary window; drain leftovers."""
            others = [[g, r, 0.0] for (g, r) in others if g is not None]
            for _ in primary:
                for o in others:
                    if o[0] is None:
                        continue
                    o[2] += o[1]
                    while o[2] >= 1.0:
                        o[2] -= 1.0
                        try:
                            next(o[0])
                        except StopIteration:
                            o[0] = None
                            break
            for o in others:
                if o[0] is not None:
                    drain(o[0])

        N_PROJ = 38.0   # approx yield counts, for pacing rates
        N_OUTP = 8.0

        drain(gen_proj(0))
        prev_c = None
        for j in range(NQ):
            n_attn = 16.0 * j + 22.0
            weave(gen_attn(j),
                  (gen_proj(j + 1) if j + 1 < NQ else None, N_PROJ / n_attn),
                  (prev_c, N_OUTP / n_attn))
            prev_c = gen_outproj(j)
        drain(prev_c)

    nc.compile()
    return nc


_EO_IDX = None


def _eo_index():
    """Column permutation within one head group: all even components of the
    4 heads first (h-major), then all odd components."""
    global _EO_IDX
    if _EO_IDX is None:
        idx = []
        for eo in (0, 1):
            for h in range(HPG):
                idx.extend(range(64 * h + eo, 64 * h + 64, 2))
        _EO_IDX = np.asarray(idx)
    return _EO_IDX


def _hilo(a):
    hi = a.astype(E4)
    lo = (a - hi.astype(np.float32)).astype(E4)
    return hi, lo


def _pack_w(w):
    """[HID, DG] f32 -> [128, KS, 2, DG] fp8 (hi/lo)."""
    hi, lo = _hilo(w)
    hi = hi.reshape(KS, 128, DG).transpose(1, 0, 2)
    lo = lo.reshape(KS, 128, DG).transpose(1, 0, 2)
    return np.ascontiguousarray(np.stack([hi, lo], axis=2))


def _pack_x(xT):
    """[HID, S] f32 -> [NQ, 128, KS, 2, 512] fp8 (hi/lo)."""
    hi, lo = _hilo(xT)
    hi = hi.reshape(KS, 128, NQ, 512)
    lo = lo.reshape(KS, 128, NQ, 512)
    pk = np.stack([hi, lo], axis=3)            # [KS, 128, NQ, 2, 512]
    return np.ascontiguousarray(pk.transpose(2, 1, 0, 3, 4))


def make_in_maps(x, Wq, bq, Wk, bk, Wv, bv, Wo, bo, mask, freqs_cos, freqs_sin):
    idx = _eo_index()
    f32 = np.float32
    cosT = np.ascontiguousarray(freqs_cos.T, dtype=f32)       # (32, S)
    sinT = np.ascontiguousarray(freqs_sin.T, dtype=f32)
    cos4 = np.tile(cosT, (4, 1)) / f32(WSCALE)                # (128, S)
    sin4 = np.tile(sinT, (4, 1)) / f32(WSCALE)

    Wq = np.asarray(Wq, f32)
    Wk = np.asarray(Wk, f32)
    Wv = np.asarray(Wv, f32)
    Wo = np.asarray(Wo, f32)
    xTs = [np.ascontiguousarray(np.asarray(x[b], f32).T) for b in range(B)]
    xpks = [_pack_x(t) for t in xTs]

    in_maps = []
    for core in range(NCORES):
        b, g = core // G, core % G
        cols = slice(DG * g, DG * (g + 1))
        wq8 = _pack_w(np.ascontiguousarray(Wq[:, cols][:, idx], f32) * WSCALE)
        wk8 = _pack_w(np.ascontiguousarray(Wk[:, cols][:, idx], f32) * WSCALE)
        wv8 = _pack_w(np.ascontiguousarray(Wv[:, cols], f32) * WSCALE)
        wo_g = np.ascontiguousarray(Wo[cols, :]).astype(BF)
        bq_g = np.ascontiguousarray(
            np.asarray(bq, f32)[cols][idx].reshape(2, 128).T) * f32(WSCALE)
        bk_g = np.ascontiguousarray(
            np.asarray(bk, f32)[cols][idx].reshape(2, 128).T) * f32(WSCALE)
        in_maps.append(dict(xpk=xpks[b], wq8=wq8, wk8=wk8, wv8=wv8, wo=wo_g,
                            bqp=bq_g, bkp=bk_g, cos4=cos4, sin4=sin4))
    return in_maps


_NC_CACHE = None
LAST_RESULTS = None


def kernel(**inputs):
    global _NC_CACHE
    if _NC_CACHE is None:
        _NC_CACHE = build_program()
    nc = _NC_CACHE

    inputs = {k: np.asarray(v) for k, v in inputs.items()}
    in_maps = make_in_maps(**inputs)
    kwargs = {}
    if os.environ.get("BASS_TRACE"):
        kwargs = dict(trace=True, trace_cores=list(range(NCORES)),
                      stitch_traces=True)
    res = run_bass_kernel_spmd(nc, in_maps, core_ids=list(range(NCORES)),
                               **kwargs)
    global LAST_RESULTS
    LAST_RESULTS = res

    out = np.zeros((B, S, HID), np.float32)
    for core in range(NCORES):
        out[core // G] += res.results[core]["out"].reshape(S, HID)
    out += inputs["bo"].astype(np.float32)
    out += (inputs["bv"].astype(np.float32) @ inputs["Wo"].astype(np.float32))
    return out


# revision 27
# speedup vs baseline: 1.1420x; 1.0076x over previous
"""Multi-head causal attention (B=2, S=2048, H=16, D=64) on 8 TRN2 NeuronCores.

Sharding: data-parallel over batch (2) x tensor-parallel over head groups (4).
Core c handles batch b = c // 4, head group g = c % 4 (heads 4g..4g+3).
Each core computes q/k/v projections for its 4 heads, RoPE, causal
flash-style attention (upper-triangular blocks skipped), and a partial
output projection out_partial = attn_out @ Wo[256g:256g+256].  The host
sums the 4 partials per batch and adds the (bias) terms.

Key layout/engine choices:
 - QKV projections run as fp8e4 DoubleRow matmuls (hi+lo split of both x
   and W computed on host; 3 of 4 cross products kept -> ~0.1% error,
   0.75x the PE cycles of fp32r and half the input DMA bytes).
 - q/k are computed TRANSPOSED (d on partitions) with W as the stationary
   operand; Wq/Wk columns are permuted to [all even | all odd] so RoPE
   runs as full-128-partition DVE ops.
 - the eo->head-contiguous regroup is a single SBUF->SBUF DMA per
   128-partition chunk: dst[2p+eo] = src[p, eo] (tandem-linear pairing),
   which interleaves each head's components as [e0 o0 e1 o1 ...] -- a
   permutation applied identically to q and k, so scores are unchanged.
 - scores/PV/out-proj matmuls run in bf16 (1 cyc/row at any N, so causal
   tail blocks are trimmed tightly to the diagonal).
 - causal masking is a gpsimd affine_select zeroing the upper triangle of
   the diagonal 128-col block of exp(scores) (no mask tensor, no DVE add;
   exp of unmasked scores is safe: |scores/8| ~ 5).
 - softmax denominators come free from a ones-column appended to v; the
   reciprocal reads that PSUM row directly and a gpsimd partition
   broadcast fans it out for the (PSUM x SBUF) normalize multiply.
 - output staging is copied PSUM->SBUF on DVE/ACT and stored from the SP
   HWDGE ring (no gpsimd SWDGE engine cost).
"""

import os
import numpy as np
import ml_dtypes
from contextlib import ExitStack

import concourse.bass as bass
import concourse.tile as tile
from concourse import bacc, mybir
from concourse.alu_op_type import AluOpType
from concourse.bass_utils import run_bass_kernel_spmd

F32 = mybir.dt.float32
BF16 = mybir.dt.bfloat16
FP8 = mybir.dt.float8e4
AF = mybir.ActivationFunctionType
DR = mybir.MatmulPerfMode.DoubleRow
E4 = ml_dtypes.float8_e4m3
BF = ml_dtypes.bfloat16

B, S, H, D = 2, 2048, 16, 64
HID = H * D           # 1024
NCORES = 8
G = 4                 # head groups
HPG = H // G          # heads per group = 4
DG = HPG * D          # per-group model dim = 256
KS = HID // 128       # 8 k-subtiles
NQ = 4                # S quarters (chunks of 512)
SB = S // 128         # 16 s-blocks

# fp8 hi-lo pairings: (w hi/lo slot, x hi/lo slot); the lo*lo term is
# dropped (~1e-3 relative contribution)
PAIRS = ((0, 0), (0, 1), (1, 0))

# W entries are ~N(0, 1/HID); scale them up so the hi-lo fp8 residual
# stays above e4m3's smallest subnormal (2^-9).  The 1/WSCALE comes out
# for free: cos/sin are pre-divided (rope multiplies by them) and the v
# copy uses the activation scale.
WSCALE = 64.0

EX_BUFS = 4
STG_BUFS = 4


def build_program():
    nc = bacc.Bacc("TRN2", target_bir_lowering=False, debug=False,
                   num_devices=NCORES)

    xpk = nc.dram_tensor("xpk", [NQ, 128, KS, 2, 512], FP8,
                         kind="ExternalInput").ap()
    wq8 = nc.dram_tensor("wq8", [128, KS, 2, DG], FP8, kind="ExternalInput").ap()
    wk8 = nc.dram_tensor("wk8", [128, KS, 2, DG], FP8, kind="ExternalInput").ap()
    wv8 = nc.dram_tensor("wv8", [128, KS, 2, DG], FP8, kind="ExternalInput").ap()
    wo = nc.dram_tensor("wo", [DG, HID], BF16, kind="ExternalInput").ap()
    bqp = nc.dram_tensor("bqp", [128, 2], F32, kind="ExternalInput").ap()
    bkp = nc.dram_tensor("bkp", [128, 2], F32, kind="ExternalInput").ap()
    cos4 = nc.dram_tensor("cos4", [128, S], F32, kind="ExternalInput").ap()
    sin4 = nc.dram_tensor("sin4", [128, S], F32, kind="ExternalInput").ap()
    out = nc.dram_tensor("out", [SB, 128, HID], F32, kind="ExternalOutput").ap()

    with tile.TileContext(nc) as tc, ExitStack() as ctx:
        const = ctx.enter_context(tc.tile_pool(name="const", bufs=1))
        xp = ctx.enter_context(tc.tile_pool(name="xp", bufs=2))
        tmp = ctx.enter_context(tc.tile_pool(name="tmp", bufs=6))
        ex = ctx.enter_context(tc.tile_pool(name="ex", bufs=EX_BUFS))
        stg = ctx.enter_context(tc.tile_pool(name="stg", bufs=STG_BUFS))
        rcp = ctx.enter_context(tc.tile_pool(name="rcp", bufs=4))
        rbp = ctx.enter_context(tc.tile_pool(name="rbp", bufs=4))
        ps = ctx.enter_context(tc.tile_pool(name="ps", bufs=2, space="PSUM"))
        psc = ctx.enter_context(tc.tile_pool(name="psc", bufs=2, space="PSUM"))
        ppv = ctx.enter_context(tc.tile_pool(name="ppv", bufs=2, space="PSUM"))

        # ---- persistent SBUF tiles ----
        wq_t = const.tile([128, KS, 2, DG], FP8)
        wk_t = const.tile([128, KS, 2, DG], FP8)
        wv_t = const.tile([128, KS, 2, DG], FP8)
        wo_t = const.tile([128, 2, HID], BF16)
        cos_t = const.tile([128, S], F32)
        sin_t = const.tile([128, S], F32)
        bq_t = const.tile([128, 2], F32)
        bk_t = const.tile([128, 2], F32)
        v1_t = const.tile([128, SB, HPG, D + 1], BF16)  # v blocks + ones col
        qr_t = const.tile([128, 2, S], BF16)   # roped q, [evens|odds] chunks
        kr_t = const.tile([128, 2, S], BF16)
        qh_t = const.tile([128, 2, S], BF16)   # head-contiguous roped q
        kh_t = const.tile([128, 2, S], BF16)
        o_t = const.tile([128, 2, S], BF16)    # attn outT (hd on partitions)

        wo_loaded = []
        outr = out  # [SB, 128, HID]

        # early loads, true dependency order (SP HWDGE ring is FIFO);
        # all weights go first so no regroup DMA can park ahead of them.
        # wq/wk come in ktile halves so the first matmuls start sooner.
        nc.sync.dma_start(wq_t[:, 0:4], wq8[:, 0:4])
        nc.sync.dma_start(bq_t[:], bqp)
        nc.sync.dma_start(wk_t[:, 0:4], wk8[:, 0:4])
        nc.sync.dma_start(bk_t[:], bkp)
        nc.sync.dma_start(wq_t[:, 4:8], wq8[:, 4:8])
        nc.sync.dma_start(wk_t[:, 4:8], wk8[:, 4:8])
        nc.sync.dma_start(wv_t[:], wv8)

        def rope_a(pc0, b_t, js):
            """t1=(e+b0)*cos, t3=(e+b0)*sin -- releases the evens psum after
            just two reads so the next projection chunk gets its bank."""
            t1 = tmp.tile([128, 512], F32, name="t1", tag="tt")
            nc.vector.scalar_tensor_tensor(t1[:], pc0[:], b_t[:, 0:1],
                                           cos_t[:, js], AluOpType.add,
                                           AluOpType.mult)
            yield
            t3 = tmp.tile([128, 512], F32, name="t3", tag="tt")
            nc.vector.scalar_tensor_tensor(t3[:], pc0[:], b_t[:, 0:1],
                                           sin_t[:, js], AluOpType.add,
                                           AluOpType.mult)
            yield
            yield (t1, t3)

        def rope_b(pc1, b_t, rr_t, js, t1, t3):
            t2 = tmp.tile([128, 512], F32, name="t2", tag="tt")
            nc.vector.scalar_tensor_tensor(t2[:], pc1[:], b_t[:, 1:2],
                                           sin_t[:, js], AluOpType.add,
                                           AluOpType.mult)
            nc.vector.tensor_sub(rr_t[:, 0, js], t1[:], t2[:])
            yield
            t4 = tmp.tile([128, 512], F32, name="t4", tag="tt")
            nc.vector.scalar_tensor_tensor(t4[:], pc1[:], b_t[:, 1:2],
                                           cos_t[:, js], AluOpType.add,
                                           AluOpType.mult)
            nc.vector.tensor_add(rr_t[:, 1, js], t3[:], t4[:])
            yield

        def fp8_proj(p_out, lhs_w, xq, c, nmax):
            """12 DoubleRow matmuls accumulating (Whi+Wlo)^T (xhi+xlo).
            ktile-half-major so only the first half of x gates the start."""
            nn = 0
            for half in range(2):
                for (wh, xh) in PAIRS:
                    for t in (2 * half, 2 * half + 1):
                        ks = slice(2 * t, 2 * t + 2)
                        nc.tensor.matmul(p_out,
                                         lhs_w[:, ks, wh, bass.ts(c, 128)],
                                         xq[:, ks, xh, :],
                                         start=(nn == 0), stop=(nn == nmax - 1),
                                         perf_mode=DR)
                        nn += 1
                    yield

        def gen_attn(j):
            """Attention for sq-quarter j; yields between pipeline units."""
            js = bass.ts(j, 512)
            if not wo_loaded:
                wo_loaded.append(1)
                nc.gpsimd.dma_start(
                    wo_t[:], wo.rearrange("(o p) n -> p o n", p=128))
            nblk = 4 * j + 4
            for cc in range(2):
                pvs = [ppv.tile([D + 1, 512], F32, name="pv", tag="pv")
                       for _ in range(2)]
                for i in range(nblk):
                    db = 128 * i - 512 * j
                    c0 = max(0, db)      # bf16: no small-N rate penalty
                    n = 512 - c0
                    spb = psc.tile([128, 2, 512], F32, name="sp", tag="sc")
                    for a in range(2):
                        hp = slice(64 * a, 64 * a + 64)
                        nc.tensor.matmul(spb[:, a, :n],
                                         kh_t[hp, cc, bass.ts(i, 128)],
                                         qh_t[hp, cc,
                                              512 * j + c0:512 * (j + 1)],
                                         start=True, stop=True)
                    et = ex.tile([128, 2, 512], BF16, name="et")
                    nc.scalar.activation(et[:, :, :n], spb[:, :, :n],
                                         AF.Exp, scale=0.125)
                    if db >= 0:
                        # zero the upper triangle of the diagonal block
                        nc.gpsimd.affine_select(
                            et[:, :, 0:128], et[:, :, 0:128],
                            pattern=[[0, 2], [1, 128]],
                            compare_op=AluOpType.is_ge, fill=0.0,
                            base=0, channel_multiplier=-1)
                    yield
                    for a in range(2):
                        nc.tensor.matmul(pvs[a][:, c0:512],
                                         v1_t[:, i, 2 * cc + a, :],
                                         et[:, a, :n],
                                         start=(i == 0), stop=(i == nblk - 1))
                    yield
                # softmax denominators live in pv rows D: reciprocal them
                # straight out of PSUM, broadcast across partitions on
                # gpsimd, then normalize pv (PSUM) x recip (SBUF) -> o_t
                rds = []
                for a in range(2):
                    rd = rcp.tile([1, 512], F32, name="rd", tag="rd")
                    nc.vector.reciprocal(rd[:], pvs[a][D:D + 1, :])
                    rds.append(rd)
                yield
                for a in range(2):
                    rb = rbp.tile([64, 512], F32, name="rb", tag="rb")
                    nc.gpsimd.partition_broadcast(rb[:], rds[a][0:1, :],
                                                  channels=64)
                    nc.vector.tensor_mul(o_t[64 * a:64 * a + 64, cc, js],
                                         pvs[a][0:D, :], rb[:])
                    yield

        def gen_outproj(j):
            for sl in range(4):
                sb = 4 * j + sl
                ps0 = ps.tile([128, 512], F32, name="psC0", tag="ps")
                ps1 = ps.tile([128, 512], F32, name="psC1", tag="ps")
                for k in range(2):
                    nc.tensor.matmul(ps0[:], o_t[:, k, bass.ts(sb, 128)],
                                     wo_t[:, k, 0:512],
                                     start=(k == 0), stop=(k == 1))
                for k in range(2):
                    nc.tensor.matmul(ps1[:], o_t[:, k, bass.ts(sb, 128)],
                                     wo_t[:, k, 512:1024],
                                     start=(k == 0), stop=(k == 1))
                st = stg.tile([128, 1024], F32, name="st")
                if j == NQ - 1:   # both ACT and DVE are idle at the tail
                    nc.scalar.activation(st[:, 0:512], ps0[:], AF.Copy)
                else:
                    nc.vector.tensor_copy(st[:, 0:512], ps0[:])
                nc.vector.tensor_copy(st[:, 512:1024], ps1[:])
                yield
                if j == NQ - 1 and sl == 3:
                    # final store: halves on two rings to shorten the tail
                    nc.scalar.dma_start(outr[sb][:, 0:512], st[:, 0:512])
                    nc.sync.dma_start(outr[sb][:, 512:1024], st[:, 512:1024])
                else:
                    nc.sync.dma_start(outr[sb], st[:])
                yield

        def gen_proj(qi):
            """fp8 projections + RoPE + head-regroup for quarter qi."""
            js = bass.ts(qi, 512)
            xq = xp.tile([128, KS, 2, 512], FP8, name="xq")
            eng = nc.scalar if qi == 0 else nc.sync
            nh = 4 if qi == 0 else 2
            for hf in range(nh):
                w = KS // nh
                eng.dma_start(xq[:, w * hf:w * hf + w],
                              xpk[qi][:, w * hf:w * hf + w])
            if qi == 0:
                nc.gpsimd.dma_start(cos_t[:], cos4)
                nc.gpsimd.dma_start(sin_t[:], sin4)
                nc.vector.memset(v1_t[:, :, :, D:D + 1], 1.0)
            def gen_v():
                # v psum tiles release right after the ACT copy, so at
                # startup (quarter 0) running v first warms the rotation
                # without waiting on rope; in woven quarters v goes last so
                # its ACT copies don't park ahead of the exp stream
                for sl in range(4):
                    sb = 4 * qi + sl
                    p = ps.tile([128, 512], F32, name="psAv", tag="ps")
                    nn = 0
                    for half in range(2):
                        for (wh, xh) in PAIRS:
                            for t in (2 * half, 2 * half + 1):
                                ks = slice(2 * t, 2 * t + 2)
                                nc.tensor.matmul(p[:, :DG],
                                                 xq[:, ks, xh,
                                                    bass.ts(sl, 128)],
                                                 wv_t[:, ks, wh, :],
                                                 start=(nn == 0),
                                                 stop=(nn == 11),
                                                 perf_mode=DR)
                                nn += 1
                            yield
                    nc.scalar.activation(v1_t[:, sb, :, 0:D],
                                         p[:, :DG].rearrange(
                                             "p (h d) -> p h d", d=D),
                                         AF.Copy, scale=1.0 / WSCALE)
                    yield

            # q/k chunks interleaved (q-c0, k-c0, q-c1, k-c1) so each psum
            # buffer's rope reads overlap the next chunk's matmuls and the
            # 2-buf rotation never stalls the PE
            ws = ((wq_t, bq_t, qr_t, qh_t), (wk_t, bk_t, kr_t, kh_t))
            t13 = [None, None]
            for c in range(2):
                for wi, (w_t, b_t, rr_t, hh_t) in enumerate(ws):
                    p = ps.tile([128, 512], F32, name="psA", tag="ps")
                    for _ in fp8_proj(p[:], w_t, xq, c, 12):
                        yield
                    if c == 0:
                        for v in rope_a(p, b_t, js):
                            if v is None:
                                yield
                            else:
                                t13[wi] = v
                    else:
                        for _ in rope_b(p, b_t, rr_t, js, *t13[wi]):
                            yield
                        # tandem-linear regroup DMA: dst[2p+eo] = src[p, eo]
                        for cc in range(2):
                            nc.sync.dma_start(hh_t[:, cc, js],
                                              rr_t[64 * cc:64 * cc + 64, :, js])
                        yield
            for _ in gen_v():
                yield

        def drain(g):
            for _ in g:
                pass

        def weave(primary, *others):
            """Emit `primary` to exhaustion, advancing each (gen, rate) in
            `others` by `rate` units per primary unit so the secondary cover
            spreads across the whole primary window; drain leftovers."""
            others = [[g, r, 0.0] for (g, r) in others if g is not None]
            for _ in primary:
                for o in others:
                    if o[0] is None:
                        continue
                    o[2] += o[1]
                    while o[2] >= 1.0:
                        o[2] -= 1.0
                        try:
                            next(o[0])
                        except StopIteration:
                            o[0] = None
                            break
            for o in others:
                if o[0] is not None:
                    drain(o[0])

        N_PROJ = 62.0   # approx yield counts, for pacing rates
        N_OUTP = 8.0

        drain(gen_proj(0))
        prev_c = None
        for j in range(NQ):
            n_attn = 16.0 * j + 22.0
            weave(gen_attn(j),
                  (gen_proj(j + 1) if j + 1 < NQ else None, N_PROJ / n_attn),
                  (prev_c, N_OUTP / n_attn))
            prev_c = gen_outproj(j)
        drain(prev_c)

    nc.compile()
    return nc


_EO_IDX = None


def _eo_index():
    """Column permutation within one head group: all even components of the
    4 heads first (h-major), then all odd components."""
    global _EO_IDX
    if _EO_IDX is None:
        idx = []
        for eo in (0, 1):
            for h in range(HPG):
                idx.extend(range(64 * h + eo, 64 * h + 64, 2))
        _EO_IDX = np.asarray(idx)
    return _EO_IDX


def _hilo(a):
    hi = a.astype(E4)
    lo = (a - hi.astype(np.float32)).astype(E4)
    return hi, lo


def _pack_w(w):
    """[HID, DG] f32 -> [128, KS, 2, DG] fp8 (hi/lo)."""
    hi, lo = _hilo(w)
    hi = hi.reshape(KS, 128, DG).transpose(1, 0, 2)
    lo = lo.reshape(KS, 128, DG).transpose(1, 0, 2)
    return np.ascontiguousarray(np.stack([hi, lo], axis=2))


def _pack_x(xT):
    """[HID, S] f32 -> [NQ, 128, KS, 2, 512] fp8 (hi/lo)."""
    hi, lo = _hilo(xT)
    hi = hi.reshape(KS, 128, NQ, 512)
    lo = lo.reshape(KS, 128, NQ, 512)
    pk = np.stack([hi, lo], axis=3)            # [KS, 128, NQ, 2, 512]
    return np.ascontiguousarray(pk.transpose(2, 1, 0, 3, 4))


def make_in_maps(x, Wq, bq, Wk, bk, Wv, bv, Wo, bo, mask, freqs_cos, freqs_sin):
    idx = _eo_index()
    f32 = np.float32
    cosT = np.ascontiguousarray(freqs_cos.T, dtype=f32)       # (32, S)
    sinT = np.ascontiguousarray(freqs_sin.T, dtype=f32)
    cos4 = np.tile(cosT, (4, 1)) / f32(WSCALE)                # (128, S)
    sin4 = np.tile(sinT, (4, 1)) / f32(WSCALE)

    Wq = np.asarray(Wq, f32)
    Wk = np.asarray(Wk, f32)
    Wv = np.asarray(Wv, f32)
    Wo = np.asarray(Wo, f32)
    xTs = [np.ascontiguousarray(np.asarray(x[b], f32).T) for b in range(B)]
    xpks = [_pack_x(t) for t in xTs]

    in_maps = []
    for core in range(NCORES):
        b, g = core // G, core % G
        cols = slice(DG * g, DG * (g + 1))
        wq8 = _pack_w(np.ascontiguousarray(Wq[:, cols][:, idx], f32) * WSCALE)
        wk8 = _pack_w(np.ascontiguousarray(Wk[:, cols][:, idx], f32) * WSCALE)
        wv8 = _pack_w(np.ascontiguousarray(Wv[:, cols], f32) * WSCALE)
        wo_g = np.ascontiguousarray(Wo[cols, :]).astype(BF)
        bq_g = np.ascontiguousarray(
            np.asarray(bq, f32)[cols][idx].reshape(2, 128).T) * f32(WSCALE)
        bk_g = np.ascontiguousarray(
            np.asarray(bk, f32)[cols][idx].reshape(2, 128).T) * f32(WSCALE)
        in_maps.append(dict(xpk=xpks[b], wq8=wq8, wk8=wk8, wv8=wv8, wo=wo_g,
                            bqp=bq_g, bkp=bk_g, cos4=cos4, sin4=sin4))
    return in_maps


_NC_CACHE = None
LAST_RESULTS = None


def kernel(**inputs):
    global _NC_CACHE
    if _NC_CACHE is None:
        _NC_CACHE = build_program()
    nc = _NC_CACHE

    inputs = {k: np.asarray(v) for k, v in inputs.items()}
    in_maps = make_in_maps(**inputs)
    kwargs = {}
    if os.environ.get("BASS_TRACE"):
        kwargs = dict(trace=True, trace_cores=list(range(NCORES)),
                      stitch_traces=True)
    res = run_bass_kernel_spmd(nc, in_maps, core_ids=list(range(NCORES)),
                               **kwargs)
    global LAST_RESULTS
    LAST_RESULTS = res

    out = np.zeros((B, S, HID), np.float32)
    for core in range(NCORES):
        out[core // G] += res.results[core]["out"].reshape(S, HID)
    out += inputs["bo"].astype(np.float32)
    out += (inputs["bv"].astype(np.float32) @ inputs["Wo"].astype(np.float32))
    return out
